# revision 1
# baseline (speedup 1.0000x reference)
"""Multi-head attention (B=2, S=2048, H=1024, NH=16, HD=64) on 8 trn2 cores.

Sharding: tensor-parallel over heads. Core c owns heads {2c, 2c+1}, i.e.
feature columns [128c, 128c+128) of q/k/v. Wq/Wk/Wv are column-sharded,
Wo row-sharded; each core computes a full-shape partial output and the
host sums the 8 partials (the row-parallel reduce) during unshard.

On-chip layout is feature-major ("transposed"): the host passes
hsT = hidden_states.T so both matmul operands of every projection have
the contraction dim on partitions and no on-chip transposes of big
tensors are needed. Attention works on scoresT[tk, tq]; softmax's
normalizer comes from a ones-column augmented V matmul (exp is safe
without max-subtraction because scores are O(6) here).

QKV and output projections run in float32r (fast fp32 mode, ~1.4e-4 rel
accuracy); score/ctx matmul operands are fp16; all accumulation is full
fp32 in PSUM. Attention matmuls are zero-padded to full 128x128 array
shapes (K=128 scores via zero-padded per-head K, M=128 ctx via padded
augmented-V) - half-array matmuls don't register as activity for the
PE's HAM clock gate and the whole phase runs at 1.2GHz otherwise.
"""

import numpy as np

B, S, H, NH, HD = 2, 2048, 1024, 16, 64
NCORES = 8
JC = 128  # head-columns per core (2 heads x 64)
T = B * S  # 4096 tokens
TQB = 512  # tq block
NKT = S // 128  # 16 tk blocks per batch
WAVE = 1024  # qkv projection token-chunk per wave
BASE = 10000.0

_nc_cache = [None]

_LDW_OPT = False


def _patch_ldw_opt():
    from concourse import bass_utils as _bu

    if getattr(_bu, "_ldw_patched", False):
        return
    _orig = _bu.run_command

    def _patched(argv, **kw):
        argv = [
            a.replace("--enable-ldw-opt=false", "--enable-ldw-opt=true")
            if _LDW_OPT and isinstance(a, str)
            else a
            for a in argv
        ]
        return _orig(argv, **kw)

    _bu.run_command = _patched
    _bu._ldw_patched = True


def _build():
    _patch_ldw_opt()
    import concourse.tile as tile
    from concourse import bacc, mybir
    from concourse.masks import make_identity

    F32 = mybir.dt.float32
    F32R = mybir.dt.float32r
    BF16 = mybir.dt.bfloat16
    F16 = mybir.dt.float16
    EXP = mybir.ActivationFunctionType.Exp

    nc = bacc.Bacc("TRN2", target_bir_lowering=False, debug=False)

    hsT = nc.dram_tensor("hsT", [H, T], F32R, kind="ExternalInput").ap()
    wqT = nc.dram_tensor("wqT", [H, JC], F32R, kind="ExternalInput").ap()
    wkT = nc.dram_tensor("wkT", [H, JC], F32R, kind="ExternalInput").ap()
    wvT = nc.dram_tensor("wvT", [H, JC], F32R, kind="ExternalInput").ap()
    woJI = nc.dram_tensor("woJI", [JC, H], F32R, kind="ExternalInput").ap()
    cosT = nc.dram_tensor("cosT", [128, S], F32, kind="ExternalInput").ap()
    sinTs = nc.dram_tensor("sinTs", [128, S], F32, kind="ExternalInput").ap()
    out = nc.dram_tensor("out", [T, H], F32, kind="ExternalOutput").ap()

    with tile.TileContext(nc) as tc:
        with (
            tc.tile_pool(name="wts", bufs=1) as wts,
            tc.tile_pool(name="tabs", bufs=1) as tabs,
            tc.tile_pool(name="hst", bufs=20) as hst,
            tc.tile_pool(name="qkv", bufs=2) as qkvp,
            tc.tile_pool(name="ps", bufs=3, space="PSUM") as ps,
            tc.tile_pool(name="cxp", bufs=2, space="PSUM") as cxp,
            tc.tile_pool(name="rope", bufs=3) as ropep,
            tc.tile_pool(name="vaug", bufs=1) as vaugp,
            tc.tile_pool(name="expt", bufs=4) as exptp,
            tc.tile_pool(name="ctx", bufs=1) as ctxp,
            tc.tile_pool(name="nrm", bufs=3) as nrmp,
            tc.tile_pool(name="outs", bufs=3) as outsp,
            tc.tile_pool(name="zdr", bufs=4, space="DRAM") as zdrp,
        ):
            # ---- persistent weights / tables ----
            wq_sb = wts.tile([128, 8, JC], F32R, tag="wq")
            nc.sync.dma_start(
                out=wq_sb[:], in_=wqT[:, :].rearrange("(k p) j -> p k j", p=128)
            )
            wk_sb = wts.tile([128, 8, JC], F32R, tag="wk")
            nc.sync.dma_start(
                out=wk_sb[:], in_=wkT[:, :].rearrange("(k p) j -> p k j", p=128)
            )
            wv_sb = wts.tile([128, 8, JC], F32R, tag="wv")
            nc.sync.dma_start(
                out=wv_sb[:], in_=wvT[:, :].rearrange("(k p) j -> p k j", p=128)
            )
            ident = tabs.tile([128, 128], F32, tag="ident")
            make_identity(nc, ident[:])
            onesc = tabs.tile([128, NKT], F32, tag="ones")
            nc.vector.memset(onesc[:], 1.0)

            for b in range(B):
                # ======== QKV projections (+RoPE), feature-major ========
                qT = qkvp.tile([128, S], F16, tag="qT")
                kT = qkvp.tile([128, S], F16, tag="kT")
                vT = qkvp.tile([128, S], F32, tag="vT")

                chains = []
                for nchi in range(S // TQB):
                    for kind, w_sb in (("q", wq_sb), ("k", wk_sb), ("v", wv_sb)):
                        chains.append((kind, w_sb, nchi))
                chunk_cache = {}

                def get_chunk(k, nchi):
                    if (k, nchi) not in chunk_cache:
                        t0 = b * S + nchi * TQB
                        c = hst.tile([128, TQB], F32R, tag="hst")
                        nc.sync.dma_start(
                            out=c[:], in_=hsT[128 * k : 128 * (k + 1), t0 : t0 + TQB]
                        )
                        chunk_cache[(k, nchi)] = c
                    return chunk_cache[(k, nchi)]

                for i0 in range(0, len(chains), 2):
                    pair = chains[i0 : i0 + 2]
                    pt_a = cxp.tile([128, TQB], F32, tag="cx")
                    pt_b = cxp.tile([128, TQB], F32, tag="cx")
                    ptiles = [pt_a, pt_b][: len(pair)]
                    for k in range(8):
                        for (kind, w_sb, nchi), p in zip(pair, ptiles):
                            nc.tensor.matmul(
                                p[:], w_sb[:, k, :], get_chunk(k, nchi)[:],
                                start=(k == 0), stop=(k == 7),
                            )
                    if b == 0 and i0 == 0:
                        cos_sb = tabs.tile([128, S], F32, tag="cos")
                        nc.sync.dma_start(out=cos_sb[:], in_=cosT[:, :])
                        sin_sb = tabs.tile([128, S], F32, tag="sin")
                        nc.sync.dma_start(out=sin_sb[:], in_=sinTs[:, :])
                    for (kind, w_sb, nchi), p in zip(pair, ptiles):
                        sl = slice(nchi * TQB, (nchi + 1) * TQB)
                        if kind == "v":
                            nc.vector.tensor_copy(vT[:, sl], p[:])
                            continue
                        dstT = qT if kind == "q" else kT
                        raw = ropep.tile([128, TQB], F32, tag="raw")
                        nc.vector.tensor_copy(raw[:], p[:])
                        rot = ropep.tile([128, TQB], F32, tag="rot")
                        for h0 in (0, 64):
                            nc.sync.dma_start(
                                out=rot[h0 : h0 + 32, :], in_=raw[h0 + 32 : h0 + 64, :]
                            )
                            nc.sync.dma_start(
                                out=rot[h0 + 32 : h0 + 64, :], in_=raw[h0 : h0 + 32, :]
                            )
                        t1 = ropep.tile([128, TQB], F32, tag="t1")
                        nc.vector.tensor_mul(t1[:], raw[:], cos_sb[:, sl])
                        t2 = ropep.tile([128, TQB], F32, tag="t2")
                        nc.vector.tensor_mul(t2[:], rot[:], sin_sb[:, sl])
                        nc.vector.tensor_add(dstT[:, sl], t1[:], t2[:])

                if b == 0:
                    wJ = wts.tile([128, H], F32R, tag="wj")
                    nc.sync.dma_start(out=wJ[:], in_=woJI[:, :])

                # zero-padded per-head K so scores run full-array K=128
                kZA = qkvp.tile([128, S], F16, tag="kZA")
                nc.vector.memset(kZA[64:128, :], 0.0)
                nc.vector.tensor_copy(kZA[0:64, :], kT[0:64, :])
                kZB = qkvp.tile([128, S], F16, tag="kZB")
                nc.vector.memset(kZB[0:64, :], 0.0)
                nc.vector.tensor_copy(kZB[64:128, :], kT[64:128, :])

                # ======== v transpose -> per-head augmented V (M padded to 128) ====
                vA = vaugp.tile([128, NKT, 128], F16, tag="vA")
                vB = vaugp.tile([128, NKT, 128], F16, tag="vB")
                nc.vector.memset(vA[:, :, 65:128], 0.0)
                nc.vector.memset(vB[:, :, 65:128], 0.0)
                nc.vector.tensor_copy(vA[:, :, 64], onesc[:])
                nc.vector.tensor_copy(vB[:, :, 64], onesc[:])
                for tkb in range(NKT):
                    pt = ps.tile([128, WAVE], F32, tag="ps")
                    nc.tensor.transpose(
                        pt[:, 0:128], vT[:, 128 * tkb : 128 * (tkb + 1)], ident[:]
                    )
                    nc.vector.tensor_copy(vA[:, tkb, 0:64], pt[:, 0:64])
                    nc.vector.tensor_copy(vB[:, tkb, 0:64], pt[:, 64:128])

                # ======== attention: scoresT -> exp -> ctxT ========
                ctxS = ctxp.tile([128, S], F32R, tag="cts")
                ctxB = ctxp.tile([64, S], F32R, tag="ctb")
                ctxA = ctxS
                for tqb in range(S // TQB):
                    qsl = slice(tqb * TQB, (tqb + 1) * TQB)
                    cxA = cxp.tile([128, TQB], F32, tag="cx")
                    cxB = cxp.tile([128, TQB], F32, tag="cx")
                    for p in range(NKT // 2):
                        scA = ps.tile([128, 2 * TQB], F32, tag="ps")
                        scB = ps.tile([128, 2 * TQB], F32, tag="ps")
                        for t in range(2):
                            tkb = 2 * p + t
                            ksl = slice(128 * tkb, 128 * (tkb + 1))
                            nc.tensor.matmul(
                                scA[:, t * TQB : (t + 1) * TQB],
                                kZA[:, ksl], qT[:, qsl],
                                start=True, stop=True,
                            )
                            nc.tensor.matmul(
                                scB[:, t * TQB : (t + 1) * TQB],
                                kZB[:, ksl], qT[:, qsl],
                                start=True, stop=True,
                            )
                        etA = exptp.tile([128, 2 * TQB], F16, tag="et")
                        nc.scalar.activation(etA[:], scA[:], EXP, scale=0.125)
                        etB = exptp.tile([128, 2 * TQB], F16, tag="et")
                        nc.scalar.activation(etB[:], scB[:], EXP, scale=0.125)
                        for t in range(2):
                            tkb = 2 * p + t
                            st, sp = tkb == 0, tkb == NKT - 1
                            tsl = slice(t * TQB, (t + 1) * TQB)
                            nc.tensor.matmul(
                                cxA[:, :], vA[:, tkb, :], etA[:, tsl],
                                start=st, stop=sp,
                            )
                            nc.tensor.matmul(
                                cxB[:, :], vB[:, tkb, :], etB[:, tsl],
                                start=st, stop=sp,
                            )
                    for cx, ctxT in ((cxA, ctxS), (cxB, ctxB)):
                        craw = nrmp.tile([65, TQB], F32, tag="craw")
                        nc.vector.tensor_copy(craw[:], cx[0:65, :])
                        rzf = nrmp.tile([1, TQB], F32, tag="rzf")
                        nc.vector.reciprocal(rzf[:], craw[64:65, :])
                        zd = zdrp.tile([1, TQB], F32, tag="zd")
                        nc.sync.dma_start(out=zd[:], in_=rzf[:])
                        zrep = nrmp.tile([64, TQB], F32, tag="zrep")
                        nc.sync.dma_start(
                            out=zrep[:], in_=zd[0:1, :].to_broadcast([64, TQB])
                        )
                        dst = ctxT[0:64, qsl] if ctxT is ctxS else ctxT[:, qsl]
                        nc.vector.tensor_mul(dst, craw[0:64, :], zrep[:])
                        if ctxT is ctxB:
                            nc.sync.dma_start(
                                out=ctxS[64:128, qsl], in_=ctxB[:, qsl]
                            )

                # ======== output projection (natural-layout out) ========
                for tq8 in range(S // 128):
                    po = ps.tile([128, WAVE], F32, tag="ps")
                    csl = slice(128 * tq8, 128 * (tq8 + 1))
                    for ich in range(2):
                        isl = slice(ich * 512, (ich + 1) * 512)
                        nc.tensor.matmul(
                            po[:, isl], ctxS[:, csl], wJ[:, isl], start=True, stop=True
                        )
                    ot = outsp.tile([128, H], F32, tag="ot")
                    if tq8 % 2 == 0:
                        nc.vector.tensor_copy(ot[:], po[:])
                    else:
                        nc.scalar.copy(ot[:], po[:])
                    nc.sync.dma_start(
                        out=out[b * S + 128 * tq8 : b * S + 128 * (tq8 + 1), :],
                        in_=ot[:],
                    )

    nc.compile()
    return nc


def _rope_tables():
    inv_freq = 1.0 / (BASE ** (np.arange(0, HD, 2, dtype=np.float64) / HD))
    t = np.arange(S, dtype=np.float64)
    freqs = np.outer(t, inv_freq)  # [S, 32]
    emb = np.concatenate([freqs, freqs], -1)  # [S, 64]
    cos = np.cos(emb).T.astype(np.float32)  # [64, S]
    sin = np.sin(emb).T.astype(np.float32)
    sin_signed = sin.copy()
    sin_signed[0:32] = -sin_signed[0:32]
    cosT = np.ascontiguousarray(np.tile(cos, (2, 1)))  # [128, S]
    sinTs = np.ascontiguousarray(np.tile(sin_signed, (2, 1)))
    return cosT, sinTs


def kernel(hidden_states, Wq, Wk, Wv, Wo):
    hidden_states = np.asarray(hidden_states, np.float32)
    Wq, Wk, Wv, Wo = (np.asarray(w, np.float32) for w in (Wq, Wk, Wv, Wo))

    if _nc_cache[0] is None:
        _nc_cache[0] = _build()
    nc = _nc_cache[0]

    hsT = np.ascontiguousarray(hidden_states.reshape(T, H).T)  # [H, T]
    cosT, sinTs = _rope_tables()
    in_maps = []
    for c in range(NCORES):
        sl = slice(JC * c, JC * (c + 1))
        in_maps.append(
            {
                "hsT": hsT,
                "wqT": np.ascontiguousarray(Wq[sl, :].T),
                "wkT": np.ascontiguousarray(Wk[sl, :].T),
                "wvT": np.ascontiguousarray(Wv[sl, :].T),
                "woJI": np.ascontiguousarray(Wo[:, sl].T),
                "cosT": cosT,
                "sinTs": sinTs,
            }
        )

    from concourse.bass_utils import run_bass_kernel_spmd

    res = run_bass_kernel_spmd(nc, in_maps, core_ids=list(range(NCORES)))
    acc = np.zeros((T, H), np.float64)
    for c in range(NCORES):
        acc += res.results[c]["out"]
    return acc.astype(np.float32).reshape(B, S, H)



# revision 15
# speedup vs baseline: 1.0572x; 1.0572x over previous
"""Multi-head attention (B=2, S=2048, H=1024, NH=16, HD=64) on 8 trn2 cores.

Sharding: tensor-parallel over heads. Core c owns heads {2c, 2c+1}, i.e.
feature columns [128c, 128c+128) of q/k/v. Wq/Wk/Wv are column-sharded,
Wo row-sharded; each core computes a full-shape partial output and the
host sums the 8 partials (the row-parallel reduce) during unshard.

On-chip layout is feature-major ("transposed"): the host passes
hsT = hidden_states.T so both matmul operands of every projection have
the contraction dim on partitions and no on-chip transposes of big
tensors are needed. Attention works on scoresT[tk, tq]; softmax's
normalizer comes from a ones-column augmented V matmul (exp is safe
without max-subtraction because scores are O(9) here).

v3 design (vs the 332us f32r baseline):
- ALL matmuls run fp16 operands (fp32 PSUM accumulate); end-to-end rel
  err ~1.7e-3 vs the 2e-2 gate (fp8 was simulated at 1.9-2.9% -
  rejected). fp16 halves LDWEIGHTS bytes and all input/output DMA.
- The attention phase is ACT-bound (128 exps x ~1.1us = 146us); the PE
  has slack there, so batch 1's QKV pair-groups are interleaved into
  batch 0's attention emission (in-order engine queues = program order
  is the schedule). Outproj for tqb N is emitted inside tqb N+1's
  p-loop so its matmuls never wait on the normalization chain. PE
  stalls also drop the HAM clock to 1.2GHz for whole 3.4us epochs, so
  keeping the PE stream dense is worth double.
- qT lives in 4 per-chunk tiles so tqb-0 scores only depend on the
  first q chunk's RoPE, not all of qT.
- Normalizer 1/Z via reciprocal_approx_fast (51 ULP, ~5x faster than
  InstReciprocal) from an SBUF staging row (PSUM input or partition-
  base-64 input to the custom-DVE op returns garbage - measured).
- Weights land pre-arranged from the host ([128, 8*128] fp16) so every
  weight DMA is contiguous.
- Attention matmuls stay zero-padded to full 128x128 array shapes
  (K=128 scores via zero-padded per-head K, M=128 ctx via padded
  augmented-V) - half-array matmuls don't register as activity for the
  PE's HAM clock gate and the whole phase runs at 1.2GHz otherwise.
"""

import numpy as np

B, S, H, NH, HD = 2, 2048, 1024, 16, 64
NCORES = 8
JC = 128  # head-columns per core (2 heads x 64)
T = B * S  # 4096 tokens
TQB = 512  # tq block
NKT = S // 128  # 16 tk blocks per batch
WAVE = 1024
BASE = 10000.0

_nc_cache = [None]

_LDW_OPT = False  # --enable-ldw-opt=true fails walrus codegen on our ldweights forms


def _patch_ldw_opt():
    from concourse import bass_utils as _bu

    if getattr(_bu, "_ldw_patched", False):
        return
    _orig = _bu.run_command

    def _patched(argv, **kw):
        argv = [
            a.replace("--enable-ldw-opt=false", "--enable-ldw-opt=true")
            if _LDW_OPT and isinstance(a, str)
            else a
            for a in argv
        ]
        return _orig(argv, **kw)

    _bu.run_command = _patched
    _bu._ldw_patched = True


def _build():
    _patch_ldw_opt()
    import concourse.tile as tile
    from concourse import bacc, mybir
    from concourse.masks import make_identity

    F32 = mybir.dt.float32
    F16 = mybir.dt.float16
    EXP = mybir.ActivationFunctionType.Exp

    nc = bacc.Bacc("TRN2", target_bir_lowering=False, debug=False)

    hsT = nc.dram_tensor("hsT", [H, T], F16, kind="ExternalInput").ap()
    wqP = nc.dram_tensor("wqP", [128, 8 * JC], F16, kind="ExternalInput").ap()
    wkP = nc.dram_tensor("wkP", [128, 8 * JC], F16, kind="ExternalInput").ap()
    wvP = nc.dram_tensor("wvP", [128, 8 * JC], F16, kind="ExternalInput").ap()
    woJI = nc.dram_tensor("woJI", [JC, H], F16, kind="ExternalInput").ap()
    cosT = nc.dram_tensor("cosT", [128, S], F16, kind="ExternalInput").ap()
    sinTs = nc.dram_tensor("sinTs", [128, S], F16, kind="ExternalInput").ap()
    out = nc.dram_tensor("out", [T, H], F16, kind="ExternalOutput").ap()

    with tile.TileContext(nc) as tc:
        with (
            tc.tile_pool(name="wts", bufs=1) as wts,
            tc.tile_pool(name="tabs", bufs=1) as tabs,
            tc.tile_pool(name="hst", bufs=36) as hst,
            tc.tile_pool(name="qkv", bufs=2) as qkvp,
            tc.tile_pool(name="ps", bufs=3, space="PSUM") as ps,
            tc.tile_pool(name="cxp", bufs=2, space="PSUM") as cxp,
            tc.tile_pool(name="rope", bufs=3) as ropep,
            tc.tile_pool(name="vaug", bufs=1) as vaugp,
            tc.tile_pool(name="expt", bufs=4) as exptp,
            tc.tile_pool(name="ctx", bufs=2) as ctxp,
            tc.tile_pool(name="nrm", bufs=4) as nrmp,
            tc.tile_pool(name="outs", bufs=3) as outsp,
            tc.tile_pool(name="zdr", bufs=4, space="DRAM") as zdrp,
        ):
            # ---- persistent weights / tables (all contiguous fp16 DMA) ----
            # wv first: the v chains run first and gate everything.
            wv_sb = wts.tile([128, 8, JC], F16, tag="wv")
            nc.sync.dma_start(
                out=wv_sb[:], in_=wvP[:, :].rearrange("p (k j) -> p k j", k=8)
            )
            wk_sb = wts.tile([128, 8, JC], F16, tag="wk")
            wq_sb = wts.tile([128, 8, JC], F16, tag="wq")
            cos_sb = tabs.tile([128, S], F16, tag="cos")
            sin_sb = tabs.tile([128, S], F16, tag="sin")
            wJ = wts.tile([128, H], F16, tag="wj")

            onesc = tabs.tile([128, NKT], F16, tag="ones")
            nc.vector.memset(onesc[:], 1.0)
            ident = tabs.tile([128, 128], F32, tag="ident")
            make_identity(nc, ident[:])

            # augmented-V and zero-padded-K buffers: one physical buffer
            # per batch parity, static zero/ones regions set once here.
            vAb, vBb, kZAb, kZBb = [], [], [], []
            for i in range(B):
                vA = vaugp.tile([128, NKT, 128], F16, tag=f"vA{i}")
                nc.vector.memset(vA[:, :, 65:128], 0.0)
                nc.vector.tensor_copy(vA[:, :, 64], onesc[:])
                vAb.append(vA)
                vB = vaugp.tile([128, NKT, 128], F16, tag=f"vB{i}")
                nc.vector.memset(vB[:, :, 65:128], 0.0)
                nc.vector.tensor_copy(vB[:, :, 64], onesc[:])
                vBb.append(vB)
                kZA = vaugp.tile([128, S], F16, tag=f"kZA{i}")
                nc.vector.memset(kZA[64:128, :], 0.0)
                kZAb.append(kZA)
                kZB = vaugp.tile([128, S], F16, tag=f"kZB{i}")
                nc.vector.memset(kZB[0:64, :], 0.0)
                kZBb.append(kZB)

            def make_qkv(b):
                """QKV projections (+RoPE) for batch b, emitted in pair-group
                steps via a generator so batch 1's groups can be interleaved
                into batch 0's (ACT-bound) attention emission."""
                vA, vB, kZA, kZB = vAb[b], vBb[b], kZAb[b], kZBb[b]
                qTc = [
                    qkvp.tile([128, TQB], F16, tag="qTc", name=f"qTc{b}_{i}", bufs=8)
                    for i in range(S // TQB)
                ]
                kT = qkvp.tile([128, S], F16, tag="kT", name=f"kT{b}")
                vT = qkvp.tile([128, S], F32, tag="vT", name=f"vT{b}")
                chunk_cache = {}

                def get_chunk(k, nchi):
                    if (k, nchi) not in chunk_cache:
                        t0 = b * S + nchi * TQB
                        c = hst.tile([128, TQB], F16, tag="hst", name="hst")
                        nc.sync.dma_start(
                            out=c[:], in_=hsT[128 * k : 128 * (k + 1), t0 : t0 + TQB]
                        )
                        chunk_cache[(k, nchi)] = c
                    return chunk_cache[(k, nchi)]

                def gen():
                    # v first (enables the v transposes early), then k, then q.
                    for kind, w_sb in (("v", wv_sb), ("k", wk_sb), ("q", wq_sb)):
                        for nch0 in range(0, S // TQB, 2):
                            pts = [
                                cxp.tile([128, TQB], F32, tag="cx", name="pt_a"),
                                cxp.tile([128, TQB], F32, tag="cx", name="pt_b"),
                            ]
                            for k in range(8):
                                for i in range(2):
                                    nc.tensor.matmul(
                                        pts[i][:],
                                        w_sb[:, k, :],
                                        get_chunk(k, nch0 + i)[:],
                                        start=(k == 0),
                                        stop=(k == 7),
                                    )
                            for i in range(2):
                                nchi = nch0 + i
                                sl = slice(nchi * TQB, (nchi + 1) * TQB)
                                p = pts[i]
                                if kind == "v":
                                    nc.scalar.copy(vT[:, sl], p[:])
                                    # PE-transpose this chunk's 4 tk blocks
                                    # into one PSUM tile, then 2 batched
                                    # copies into the fp16 augmented-V layout.
                                    tp = cxp.tile(
                                        [128, TQB], F32, tag="cx", name="tp"
                                    )
                                    for j in range(TQB // 128):
                                        nc.tensor.transpose(
                                            tp[:, 128 * j : 128 * (j + 1)],
                                            vT[:, sl][:, 128 * j : 128 * (j + 1)],
                                            ident[:],
                                        )
                                    tpv = tp[:].rearrange("p (j c) -> p j c", j=4)
                                    t0 = nchi * (TQB // 128)
                                    nc.vector.tensor_copy(
                                        vA[:, t0 : t0 + 4, 0:64], tpv[:, :, 0:64]
                                    )
                                    nc.vector.tensor_copy(
                                        vB[:, t0 : t0 + 4, 0:64], tpv[:, :, 64:128]
                                    )
                                    continue
                                raw = ropep.tile([128, TQB], F16, tag="raw")
                                nc.scalar.copy(raw[:], p[:])
                                rot = ropep.tile([128, TQB], F16, tag="rot")
                                for h0 in (0, 64):
                                    nc.sync.dma_start(
                                        out=rot[h0 : h0 + 32, :],
                                        in_=raw[h0 + 32 : h0 + 64, :],
                                    )
                                    nc.sync.dma_start(
                                        out=rot[h0 + 32 : h0 + 64, :],
                                        in_=raw[h0 : h0 + 32, :],
                                    )
                                t1 = ropep.tile([128, TQB], F16, tag="t1")
                                nc.vector.tensor_mul(t1[:], raw[:], cos_sb[:, sl])
                                t2 = ropep.tile([128, TQB], F16, tag="t2")
                                nc.vector.tensor_mul(t2[:], rot[:], sin_sb[:, sl])
                                if kind == "q":
                                    nc.vector.tensor_add(qTc[nchi][:], t1[:], t2[:])
                                else:
                                    nc.vector.tensor_add(kT[:, sl], t1[:], t2[:])
                            yield
                        if kind == "k":
                            nc.vector.tensor_copy(kZA[0:64, :], kT[0:64, :])
                            nc.vector.tensor_copy(kZB[64:128, :], kT[64:128, :])

                return gen(), qTc

            def attention(b, qTc, ileave):
                """scoresT -> exp -> ctxT -> normalize, with outproj for tqb N
                emitted inside tqb N+1's p-loop (so its PE matmuls never wait
                on the normalization chain), and the next batch's QKV
                pair-groups interleaved at p==2/p==5 (PE has slack under the
                ACT-bound exp stream)."""
                vA, vB, kZA, kZB = vAb[b], vBb[b], kZAb[b], kZBb[b]
                ctxS = ctxp.tile([128, S], F16, tag="cts", name=f"ctxS{b}")
                ctxB = ctxp.tile([64, S], F16, tag="ctb", name=f"ctxB{b}")

                def emit_outproj(tqb):
                    for j8 in range(TQB // 128):
                        tq8 = tqb * (TQB // 128) + j8
                        po = ps.tile([128, WAVE], F32, tag="ps", name="po")
                        csl = slice(128 * tq8, 128 * (tq8 + 1))
                        for ich in range(2):
                            isl = slice(ich * 512, (ich + 1) * 512)
                            nc.tensor.matmul(
                                po[:, isl], ctxS[:, csl], wJ[:, isl],
                                start=True, stop=True,
                            )
                        ot = outsp.tile([128, H], F16, tag="ot")
                        if tq8 % 2 == 0:
                            nc.vector.tensor_copy(ot[:], po[:])
                        else:
                            nc.scalar.copy(ot[:], po[:])
                        nc.scalar.dma_start(
                            out=out[b * S + 128 * tq8 : b * S + 128 * (tq8 + 1), :],
                            in_=ot[:],
                        )

                prev_tqb = None
                for tqb in range(S // TQB):
                    qsl = slice(tqb * TQB, (tqb + 1) * TQB)
                    cxA = cxp.tile([128, TQB], F32, tag="cx")
                    cxB = cxp.tile([128, TQB], F32, tag="cx")
                    for p in range(NKT // 2):
                        if p in (2, 5) and ileave is not None:
                            next(ileave, None)
                        if p == NKT // 2 - 1 and prev_tqb is not None:
                            emit_outproj(prev_tqb)
                            prev_tqb = None
                        scA = ps.tile([128, 2 * TQB], F32, tag="ps")
                        scB = ps.tile([128, 2 * TQB], F32, tag="ps")
                        for t in range(2):
                            tkb = 2 * p + t
                            ksl = slice(128 * tkb, 128 * (tkb + 1))
                            nc.tensor.matmul(
                                scA[:, t * TQB : (t + 1) * TQB],
                                kZA[:, ksl], qTc[tqb][:],
                                start=True, stop=True,
                            )
                            nc.tensor.matmul(
                                scB[:, t * TQB : (t + 1) * TQB],
                                kZB[:, ksl], qTc[tqb][:],
                                start=True, stop=True,
                            )
                        etA = exptp.tile([128, 2 * TQB], F16, tag="et")
                        nc.scalar.activation(etA[:], scA[:], EXP, scale=0.125)
                        etB = exptp.tile([128, 2 * TQB], F16, tag="et")
                        nc.scalar.activation(etB[:], scB[:], EXP, scale=0.125)
                        for t in range(2):
                            tkb = 2 * p + t
                            st, sp = tkb == 0, tkb == NKT - 1
                            tsl = slice(t * TQB, (t + 1) * TQB)
                            nc.tensor.matmul(
                                cxA[:, :], vA[:, tkb, :], etA[:, tsl],
                                start=st, stop=sp,
                            )
                            nc.tensor.matmul(
                                cxB[:, :], vB[:, tkb, :], etB[:, tsl],
                                start=st, stop=sp,
                            )
                    for cx, ctxT in ((cxA, ctxS), (cxB, ctxB)):
                        zrow = nrmp.tile([1, TQB], F32, tag="zrow")
                        nc.vector.tensor_copy(zrow[:], cx[64:65, :])
                        rzf = nrmp.tile([1, TQB], F32, tag="rzf")
                        nc.vector.reciprocal_approx_fast(out=rzf[:], in_=zrow[:])
                        zd = zdrp.tile([1, TQB], F32, tag="zd")
                        nc.sync.dma_start(out=zd[:], in_=rzf[:])
                        zrep = nrmp.tile([64, TQB], F32, tag="zrep")
                        nc.sync.dma_start(
                            out=zrep[:], in_=zd[0:1, :].to_broadcast([64, TQB])
                        )
                        dst = ctxT[0:64, qsl] if ctxT is ctxS else ctxT[:, qsl]
                        nc.vector.tensor_mul(dst, cx[0:64, :], zrep[:])
                        if ctxT is ctxB:
                            nc.scalar.dma_start(
                                out=ctxS[64:128, qsl], in_=ctxB[:, qsl]
                            )
                    prev_tqb = tqb
                emit_outproj(prev_tqb)
                if ileave is not None:
                    for _ in ileave:
                        pass

            gen0, qTc0 = make_qkv(0)
            # queue the first v pair-group (and its chunk DMAs) before the
            # remaining weight/table loads so the PE starts ASAP.
            next(gen0, None)
            nc.sync.dma_start(
                out=wk_sb[:], in_=wkP[:, :].rearrange("p (k j) -> p k j", k=8)
            )
            nc.sync.dma_start(
                out=wq_sb[:], in_=wqP[:, :].rearrange("p (k j) -> p k j", k=8)
            )
            nc.scalar.dma_start(out=cos_sb[:], in_=cosT[:, :])
            nc.scalar.dma_start(out=sin_sb[:], in_=sinTs[:, :])
            nc.scalar.dma_start(out=wJ[:], in_=woJI[:, :])
            for _ in gen0:
                pass
            gen1, qTc1 = make_qkv(1)
            attention(0, qTc0, gen1)
            attention(1, qTc1, None)

    nc.compile()
    return nc


def _rope_tables():
    inv_freq = 1.0 / (BASE ** (np.arange(0, HD, 2, dtype=np.float64) / HD))
    t = np.arange(S, dtype=np.float64)
    freqs = np.outer(t, inv_freq)  # [S, 32]
    emb = np.concatenate([freqs, freqs], -1)  # [S, 64]
    cos = np.cos(emb).T.astype(np.float32)  # [64, S]
    sin = np.sin(emb).T.astype(np.float32)
    sin_signed = sin.copy()
    sin_signed[0:32] = -sin_signed[0:32]
    cosT = np.ascontiguousarray(np.tile(cos, (2, 1))).astype(np.float16)  # [128, S]
    sinTs = np.ascontiguousarray(np.tile(sin_signed, (2, 1))).astype(np.float16)
    return cosT, sinTs


def _make_in_maps(hidden_states, Wq, Wk, Wv, Wo):
    hsT = np.ascontiguousarray(
        hidden_states.astype(np.float32).reshape(T, H).T.astype(np.float16)
    )  # [H, T]
    cosT, sinTs = _rope_tables()

    def pack_w(Wx, sl):
        # SBUF layout [p, k, j]: partition p holds rows {k*128+p} of Wx[sl,:].T
        wT = Wx[sl, :].astype(np.float16).T  # [H, JC]
        return np.ascontiguousarray(
            wT.reshape(8, 128, JC).transpose(1, 0, 2).reshape(128, 8 * JC)
        )

    in_maps = []
    for c in range(NCORES):
        sl = slice(JC * c, JC * (c + 1))
        in_maps.append(
            {
                "hsT": hsT,
                "wqP": pack_w(Wq, sl),
                "wkP": pack_w(Wk, sl),
                "wvP": pack_w(Wv, sl),
                "woJI": np.ascontiguousarray(Wo[:, sl].astype(np.float16).T),
                "cosT": cosT,
                "sinTs": sinTs,
            }
        )
    return in_maps


def kernel(hidden_states, Wq, Wk, Wv, Wo):
    hidden_states = np.asarray(hidden_states, np.float32)
    Wq, Wk, Wv, Wo = (np.asarray(w, np.float32) for w in (Wq, Wk, Wv, Wo))

    if _nc_cache[0] is None:
        _nc_cache[0] = _build()
    nc = _nc_cache[0]

    in_maps = _make_in_maps(hidden_states, Wq, Wk, Wv, Wo)

    from concourse.bass_utils import run_bass_kernel_spmd

    res = run_bass_kernel_spmd(nc, in_maps, core_ids=list(range(NCORES)))
    acc = np.zeros((T, H), np.float32)
    for c in range(NCORES):
        acc += res.results[c]["out"].astype(np.float32)
    return acc.reshape(B, S, H)


# revision 17
# speedup vs baseline: 1.1422x; 1.0804x over previous
"""Multi-head attention (B=2, S=2048, H=1024, NH=16, HD=64) on 8 trn2 cores.

Sharding: tensor-parallel over heads. Core c owns heads {2c, 2c+1}, i.e.
feature columns [128c, 128c+128) of q/k/v. Wq/Wk/Wv are column-sharded,
Wo row-sharded; each core computes a full-shape partial output and the
host sums the 8 partials (the row-parallel reduce) during unshard.

On-chip layout is feature-major ("transposed"): the host passes
hsT = hidden_states.T so both matmul operands of every projection have
the contraction dim on partitions and no on-chip transposes of big
tensors are needed. Attention works on scoresT[tk, tq]; softmax's
normalizer comes from a ones-column augmented V matmul (exp is safe
without max-subtraction because scores are O(9) here).

v4 design (vs the 332us f32r baseline):
- ALL matmuls run fp16 operands (fp32 PSUM accumulate); end-to-end rel
  err ~1.7e-3 vs the 2e-2 gate (fp8 simulated at 1.9-2.9% - rejected).
- The attention phase is ACT-bound (128 exps x ~1.14us); everything
  else is kept OFF the ACT engine during attention, and the PE's slack
  is filled by interleaving batch 1's QKV pair-groups plus the
  previous tqb's output projection into the attention emission stream
  (in-order engine queues = program order is the schedule). PE stalls
  also drop the HAM clock to 1.2GHz for whole 3.4us epochs, so a dense
  PE stream is worth double.
- PSUM budget (8 banks): scores 2x[128,1024] (4) + ctx accumulators
  2x[128,512] (2) + transient ring 2x[128,512] (2) shared by QKV
  projection pairs, v-transposes and outproj halves. The ctx
  accumulators are freed immediately after a craw/zrow SBUF copy; the
  1/Z chain (reciprocal_approx_fast + DRAM broadcast bounce) and the
  normalize-multiply are emitted a tqb later so their DMA latency
  hides under the exp stream.
- reciprocal_approx_fast must read its input from an SBUF tile at
  partition base 0 (PSUM or partition-base-64 input returns garbage -
  measured on hw).
- GpSimd runs the rope sin-mul/add, kZ copies, normalize-muls and half
  the outproj PSUM->SBUF copies; DVE takes the rest.
- Weights land pre-arranged from the host ([128, 8*128] fp16) so every
  weight DMA is contiguous; qT lives in 4 per-chunk tiles so tqb-0
  scores only depend on the first q chunk's RoPE.
- Attention matmuls stay zero-padded to full 128x128 array shapes
  (K=128 scores via zero-padded per-head K, M=128 ctx via padded
  augmented-V) - half-array matmuls don't register as activity for the
  PE's HAM clock gate and the whole phase runs at 1.2GHz otherwise.
"""

import numpy as np

B, S, H, NH, HD = 2, 2048, 1024, 16, 64
NCORES = 8
JC = 128  # head-columns per core (2 heads x 64)
T = B * S  # 4096 tokens
TQB = 512  # tq block
NKT = S // 128  # 16 tk blocks per batch
WAVE = 1024
BASE = 10000.0

_nc_cache = [None]

_LDW_OPT = False  # --enable-ldw-opt=true fails walrus codegen on our ldweights forms
_POOL_PSUM = False  # GPSIMD cannot access PSUM (BIR verifier)
_POOL_ROPE = True  # gpsimd does the rope sin-mul + add


def _patch_ldw_opt():
    from concourse import bass_utils as _bu

    if getattr(_bu, "_ldw_patched", False):
        return
    _orig = _bu.run_command

    def _patched(argv, **kw):
        argv = [
            a.replace("--enable-ldw-opt=false", "--enable-ldw-opt=true")
            if _LDW_OPT and isinstance(a, str)
            else a
            for a in argv
        ]
        return _orig(argv, **kw)

    _bu.run_command = _patched
    _bu._ldw_patched = True


def _build():
    _patch_ldw_opt()
    import concourse.tile as tile
    from concourse import bacc, mybir
    from concourse.masks import make_identity

    F32 = mybir.dt.float32
    F16 = mybir.dt.float16
    EXP = mybir.ActivationFunctionType.Exp

    nc = bacc.Bacc("TRN2", target_bir_lowering=False, debug=False)

    hsT = nc.dram_tensor("hsT", [H, T], F16, kind="ExternalInput").ap()
    wqP = nc.dram_tensor("wqP", [128, 8 * JC], F16, kind="ExternalInput").ap()
    wkP = nc.dram_tensor("wkP", [128, 8 * JC], F16, kind="ExternalInput").ap()
    wvP = nc.dram_tensor("wvP", [128, 8 * JC], F16, kind="ExternalInput").ap()
    woJI = nc.dram_tensor("woJI", [JC, H], F16, kind="ExternalInput").ap()
    cosT = nc.dram_tensor("cosT", [128, S], F16, kind="ExternalInput").ap()
    sinTs = nc.dram_tensor("sinTs", [128, S], F16, kind="ExternalInput").ap()
    out = nc.dram_tensor("out", [T, H], F16, kind="ExternalOutput").ap()

    with tile.TileContext(nc) as tc:
        with (
            tc.tile_pool(name="wts", bufs=1) as wts,
            tc.tile_pool(name="tabs", bufs=1) as tabs,
            tc.tile_pool(name="hst", bufs=36) as hst,
            tc.tile_pool(name="qkv", bufs=2) as qkvp,
            tc.tile_pool(name="ps", bufs=2, space="PSUM") as ps,
            tc.tile_pool(name="cxp", bufs=2, space="PSUM") as cxp,
            tc.tile_pool(name="rope", bufs=3) as ropep,
            tc.tile_pool(name="vaug", bufs=1) as vaugp,
            tc.tile_pool(name="expt", bufs=4) as exptp,
            tc.tile_pool(name="ctx", bufs=2) as ctxp,
            tc.tile_pool(name="nrm", bufs=4) as nrmp,
            tc.tile_pool(name="outs", bufs=4) as outsp,
            tc.tile_pool(name="zdr", bufs=4, space="DRAM") as zdrp,
        ):
            # ---- persistent weights / tables (all contiguous fp16 DMA) ----
            # wv first: the v chains run first and gate everything.
            wv_sb = wts.tile([128, 8, JC], F16, tag="wv")
            nc.sync.dma_start(
                out=wv_sb[:], in_=wvP[:, :].rearrange("p (k j) -> p k j", k=8)
            )
            wk_sb = wts.tile([128, 8, JC], F16, tag="wk")
            wq_sb = wts.tile([128, 8, JC], F16, tag="wq")
            cos_sb = tabs.tile([128, S], F16, tag="cos")
            sin_sb = tabs.tile([128, S], F16, tag="sin")
            wJ = wts.tile([128, H], F16, tag="wj")

            onesc = tabs.tile([128, NKT], F16, tag="ones")
            nc.vector.memset(onesc[:], 1.0)
            ident = tabs.tile([128, 128], F32, tag="ident")
            make_identity(nc, ident[:])

            # augmented-V and zero-padded-K buffers: one physical buffer
            # per batch parity, static zero/ones regions set once here.
            vAb, vBb, kZAb, kZBb = [], [], [], []
            for i in range(B):
                vA = vaugp.tile([128, NKT, 128], F16, tag=f"vA{i}")
                nc.vector.memset(vA[:, :, 65:128], 0.0)
                nc.vector.tensor_copy(vA[:, :, 64], onesc[:])
                vAb.append(vA)
                vB = vaugp.tile([128, NKT, 128], F16, tag=f"vB{i}")
                nc.vector.memset(vB[:, :, 65:128], 0.0)
                nc.vector.tensor_copy(vB[:, :, 64], onesc[:])
                vBb.append(vB)
                kZA = vaugp.tile([128, S], F16, tag=f"kZA{i}")
                nc.vector.memset(kZA[64:128, :], 0.0)
                kZAb.append(kZA)
                kZB = vaugp.tile([128, S], F16, tag=f"kZB{i}")
                nc.vector.memset(kZB[0:64, :], 0.0)
                kZBb.append(kZB)

            kzeng = nc.gpsimd if _POOL_ROPE else nc.vector

            def make_qkv(b):
                """QKV projections (+RoPE) for batch b, emitted in pair-group
                steps via a generator so batch 1's groups can be interleaved
                into batch 0's (ACT-bound) attention emission. Keeps the ACT
                engine untouched: PSUM->SBUF copies go to DVE, the rope
                sin-mul/add to GpSimd."""
                vA, vB, kZA, kZB = vAb[b], vBb[b], kZAb[b], kZBb[b]
                qTc = [
                    qkvp.tile([128, TQB], F16, tag="qTc", name=f"qTc{b}_{i}", bufs=8)
                    for i in range(S // TQB)
                ]
                kT = qkvp.tile([128, S], F16, tag="kT", name=f"kT{b}")
                vT = qkvp.tile([128, S], F32, tag="vT", name=f"vT{b}")
                chunk_cache = {}

                def get_chunk(k, nchi):
                    if (k, nchi) not in chunk_cache:
                        t0 = b * S + nchi * TQB
                        c = hst.tile([128, TQB], F16, tag="hst", name="hst")
                        nc.sync.dma_start(
                            out=c[:], in_=hsT[128 * k : 128 * (k + 1), t0 : t0 + TQB]
                        )
                        chunk_cache[(k, nchi)] = c
                    return chunk_cache[(k, nchi)]

                def gen():
                    # v first (enables the v transposes early), then k, then q.
                    for kind, w_sb in (("v", wv_sb), ("k", wk_sb), ("q", wq_sb)):
                        for nch0 in range(0, S // TQB, 2):
                            pts = [
                                cxp.tile([128, TQB], F32, tag="tr", name="pt_a"),
                                cxp.tile([128, TQB], F32, tag="tr", name="pt_b"),
                            ]
                            for k in range(8):
                                for i in range(2):
                                    nc.tensor.matmul(
                                        pts[i][:],
                                        w_sb[:, k, :],
                                        get_chunk(k, nch0 + i)[:],
                                        start=(k == 0),
                                        stop=(k == 7),
                                    )
                            for i in range(2):
                                nchi = nch0 + i
                                sl = slice(nchi * TQB, (nchi + 1) * TQB)
                                p = pts[i]
                                if kind == "v":
                                    nc.vector.tensor_copy(vT[:, sl], p[:])
                                    # PE-transpose this chunk's 4 tk blocks
                                    # into one PSUM tile, then 2 batched
                                    # copies into the fp16 augmented-V layout.
                                    tp = cxp.tile(
                                        [128, TQB], F32, tag="tr", name="tp"
                                    )
                                    for j in range(TQB // 128):
                                        nc.tensor.transpose(
                                            tp[:, 128 * j : 128 * (j + 1)],
                                            vT[:, sl][:, 128 * j : 128 * (j + 1)],
                                            ident[:],
                                        )
                                    tpv = tp[:].rearrange("p (j c) -> p j c", j=4)
                                    t0 = nchi * (TQB // 128)
                                    nc.vector.tensor_copy(
                                        vA[:, t0 : t0 + 4, 0:64], tpv[:, :, 0:64]
                                    )
                                    nc.vector.tensor_copy(
                                        vB[:, t0 : t0 + 4, 0:64], tpv[:, :, 64:128]
                                    )
                                    continue
                                raw = ropep.tile([128, TQB], F16, tag="raw")
                                nc.vector.tensor_copy(raw[:], p[:])
                                rot = ropep.tile([128, TQB], F16, tag="rot")
                                for h0 in (0, 64):
                                    nc.sync.dma_start(
                                        out=rot[h0 : h0 + 32, :],
                                        in_=raw[h0 + 32 : h0 + 64, :],
                                    )
                                    nc.sync.dma_start(
                                        out=rot[h0 + 32 : h0 + 64, :],
                                        in_=raw[h0 : h0 + 32, :],
                                    )
                                t1 = ropep.tile([128, TQB], F16, tag="t1")
                                nc.vector.tensor_mul(t1[:], raw[:], cos_sb[:, sl])
                                t2 = ropep.tile([128, TQB], F16, tag="t2")
                                kzeng.tensor_mul(t2[:], rot[:], sin_sb[:, sl])
                                dst = qTc[nchi][:] if kind == "q" else kT[:, sl]
                                kzeng.tensor_add(dst, t1[:], t2[:])
                            yield
                        if kind == "k":
                            kzeng.tensor_copy(kZA[0:64, :], kT[0:64, :])
                            kzeng.tensor_copy(kZB[64:128, :], kT[64:128, :])

                return gen(), qTc

            def attention(b, qTc, ileave):
                """scoresT -> exp -> ctxT -> normalize. The p-loop emits, at
                fixed p slots: next batch's QKV pair-groups (p 2/5), the 1/Z
                finish + normalize-mul of the previous tqb (p 1), and the
                previous tqb's output projection (p 7). The ACT engine sees
                nothing but the 16 exps per tqb."""
                vA, vB, kZA, kZB = vAb[b], vBb[b], kZAb[b], kZBb[b]
                ctxS = ctxp.tile([128, S], F16, tag="cts", name=f"ctxS{b}")
                ctxB = ctxp.tile([64, S], F16, tag="ctb", name=f"ctxB{b}")
                pend = {}  # tqb -> (crawA, zrepA, crawB, zrepB)

                def norm_phase1(tqb, cxA, cxB):
                    # free the ctx PSUM accumulators ASAP: copy out rows 0:64
                    # (craw) + row 64 (zrow), then 1/Z + DRAM broadcast bounce.
                    res = []
                    for cx in (cxA, cxB):
                        craw = nrmp.tile([64, TQB], F32, tag="craw")
                        nc.vector.tensor_copy(craw[:], cx[0:64, :])
                        zrow = nrmp.tile([1, TQB], F32, tag="zrow")
                        nc.vector.tensor_copy(zrow[:], cx[64:65, :])
                        rzf = nrmp.tile([1, TQB], F32, tag="rzf")
                        nc.vector.reciprocal_approx_fast(out=rzf[:], in_=zrow[:])
                        zd = zdrp.tile([1, TQB], F32, tag="zd")
                        nc.sync.dma_start(out=zd[:], in_=rzf[:])
                        zrep = nrmp.tile([64, TQB], F32, tag="zrep")
                        nc.sync.dma_start(
                            out=zrep[:], in_=zd[0:1, :].to_broadcast([64, TQB])
                        )
                        res += [craw, zrep]
                    pend[tqb] = res

                def norm_phase2(tqb):
                    crawA, zrepA, crawB, zrepB = pend.pop(tqb)
                    qsl = slice(tqb * TQB, (tqb + 1) * TQB)
                    nc.gpsimd.tensor_mul(ctxS[0:64, qsl], crawA[:], zrepA[:])
                    nc.gpsimd.tensor_mul(ctxB[:, qsl], crawB[:], zrepB[:])
                    nc.scalar.dma_start(out=ctxS[64:128, qsl], in_=ctxB[:, qsl])

                def emit_outproj(tqb):
                    for j8 in range(TQB // 128):
                        tq8 = tqb * (TQB // 128) + j8
                        csl = slice(128 * tq8, 128 * (tq8 + 1))
                        poa = cxp.tile([128, TQB], F32, tag="tr", name="poa")
                        pob = cxp.tile([128, TQB], F32, tag="tr", name="pob")
                        nc.tensor.matmul(
                            poa[:], ctxS[:, csl], wJ[:, 0:512],
                            start=True, stop=True,
                        )
                        nc.tensor.matmul(
                            pob[:], ctxS[:, csl], wJ[:, 512:1024],
                            start=True, stop=True,
                        )
                        ot = outsp.tile([128, H], F16, tag="ot")
                        nc.vector.tensor_copy(ot[:, 0:512], poa[:])
                        oeng = nc.gpsimd if _POOL_PSUM else nc.vector
                        oeng.tensor_copy(ot[:, 512:1024], pob[:])
                        nc.scalar.dma_start(
                            out=out[b * S + 128 * tq8 : b * S + 128 * (tq8 + 1), :],
                            in_=ot[:],
                        )

                prev_tqb = None
                for tqb in range(S // TQB):
                    cxA = cxp.tile([128, TQB], F32, tag="cx")
                    cxB = cxp.tile([128, TQB], F32, tag="cx")
                    for p in range(NKT // 2):
                        if p == 1 and prev_tqb is not None:
                            norm_phase2(prev_tqb)
                        if p in (2, 5) and ileave is not None:
                            next(ileave, None)
                        if p == NKT // 2 - 1 and prev_tqb is not None:
                            emit_outproj(prev_tqb)
                            prev_tqb = None
                        scA = ps.tile([128, 2 * TQB], F32, tag="ps")
                        scB = ps.tile([128, 2 * TQB], F32, tag="ps")
                        for t in range(2):
                            tkb = 2 * p + t
                            ksl = slice(128 * tkb, 128 * (tkb + 1))
                            nc.tensor.matmul(
                                scA[:, t * TQB : (t + 1) * TQB],
                                kZA[:, ksl], qTc[tqb][:],
                                start=True, stop=True,
                            )
                            nc.tensor.matmul(
                                scB[:, t * TQB : (t + 1) * TQB],
                                kZB[:, ksl], qTc[tqb][:],
                                start=True, stop=True,
                            )
                        etA = exptp.tile([128, 2 * TQB], F16, tag="et")
                        nc.scalar.activation(etA[:], scA[:], EXP, scale=0.125)
                        etB = exptp.tile([128, 2 * TQB], F16, tag="et")
                        nc.scalar.activation(etB[:], scB[:], EXP, scale=0.125)
                        for t in range(2):
                            tkb = 2 * p + t
                            st, sp = tkb == 0, tkb == NKT - 1
                            tsl = slice(t * TQB, (t + 1) * TQB)
                            nc.tensor.matmul(
                                cxA[:, :], vA[:, tkb, :], etA[:, tsl],
                                start=st, stop=sp,
                            )
                            nc.tensor.matmul(
                                cxB[:, :], vB[:, tkb, :], etB[:, tsl],
                                start=st, stop=sp,
                            )
                    norm_phase1(tqb, cxA, cxB)
                    prev_tqb = tqb
                norm_phase2(prev_tqb)
                emit_outproj(prev_tqb)
                if ileave is not None:
                    for _ in ileave:
                        pass

            gen0, qTc0 = make_qkv(0)
            # queue the first v pair-group (and its chunk DMAs) before the
            # remaining weight/table loads so the PE starts ASAP.
            next(gen0, None)
            nc.sync.dma_start(
                out=wk_sb[:], in_=wkP[:, :].rearrange("p (k j) -> p k j", k=8)
            )
            nc.sync.dma_start(
                out=wq_sb[:], in_=wqP[:, :].rearrange("p (k j) -> p k j", k=8)
            )
            nc.scalar.dma_start(out=cos_sb[:], in_=cosT[:, :])
            nc.scalar.dma_start(out=sin_sb[:], in_=sinTs[:, :])
            nc.scalar.dma_start(out=wJ[:], in_=woJI[:, :])
            for _ in gen0:
                pass
            gen1, qTc1 = make_qkv(1)
            attention(0, qTc0, gen1)
            attention(1, qTc1, None)

    nc.compile()
    return nc


def _rope_tables():
    inv_freq = 1.0 / (BASE ** (np.arange(0, HD, 2, dtype=np.float64) / HD))
    t = np.arange(S, dtype=np.float64)
    freqs = np.outer(t, inv_freq)  # [S, 32]
    emb = np.concatenate([freqs, freqs], -1)  # [S, 64]
    cos = np.cos(emb).T.astype(np.float32)  # [64, S]
    sin = np.sin(emb).T.astype(np.float32)
    sin_signed = sin.copy()
    sin_signed[0:32] = -sin_signed[0:32]
    cosT = np.ascontiguousarray(np.tile(cos, (2, 1))).astype(np.float16)  # [128, S]
    sinTs = np.ascontiguousarray(np.tile(sin_signed, (2, 1))).astype(np.float16)
    return cosT, sinTs


def _make_in_maps(hidden_states, Wq, Wk, Wv, Wo):
    hsT = np.ascontiguousarray(
        hidden_states.astype(np.float32).reshape(T, H).T.astype(np.float16)
    )  # [H, T]
    cosT, sinTs = _rope_tables()

    def pack_w(Wx, sl):
        # SBUF layout [p, k, j]: partition p holds rows {k*128+p} of Wx[sl,:].T
        wT = Wx[sl, :].astype(np.float16).T  # [H, JC]
        return np.ascontiguousarray(
            wT.reshape(8, 128, JC).transpose(1, 0, 2).reshape(128, 8 * JC)
        )

    in_maps = []
    for c in range(NCORES):
        sl = slice(JC * c, JC * (c + 1))
        in_maps.append(
            {
                "hsT": hsT,
                "wqP": pack_w(Wq, sl),
                "wkP": pack_w(Wk, sl),
                "wvP": pack_w(Wv, sl),
                "woJI": np.ascontiguousarray(Wo[:, sl].astype(np.float16).T),
                "cosT": cosT,
                "sinTs": sinTs,
            }
        )
    return in_maps


def kernel(hidden_states, Wq, Wk, Wv, Wo):
    hidden_states = np.asarray(hidden_states, np.float32)
    Wq, Wk, Wv, Wo = (np.asarray(w, np.float32) for w in (Wq, Wk, Wv, Wo))

    if _nc_cache[0] is None:
        _nc_cache[0] = _build()
    nc = _nc_cache[0]

    in_maps = _make_in_maps(hidden_states, Wq, Wk, Wv, Wo)

    from concourse.bass_utils import run_bass_kernel_spmd

    res = run_bass_kernel_spmd(nc, in_maps, core_ids=list(range(NCORES)))
    acc = np.zeros((T, H), np.float32)
    for c in range(NCORES):
        acc += res.results[c]["out"].astype(np.float32)
    return acc.reshape(B, S, H)


# revision 23
# speedup vs baseline: 1.2423x; 1.0876x over previous
"""Multi-head attention (B=2, S=2048, H=1024, NH=16, HD=64) on 8 trn2 cores.

Sharding: tensor-parallel over heads. Core c owns heads {2c, 2c+1}, i.e.
feature columns [128c, 128c+128) of q/k/v. Wq/Wk/Wv are column-sharded,
Wo row-sharded; each core computes a full-shape partial output and the
host sums the 8 partials (the row-parallel reduce) during unshard.

On-chip layout is feature-major ("transposed"): the host passes
hsT = hidden_states.T so both matmul operands of every projection have
the contraction dim on partitions and no on-chip transposes of big
tensors are needed. Attention works on scoresT[tk, tq]; softmax's
normalizer comes from a ones-column augmented V matmul (exp is safe
without max-subtraction because scores are O(9) here).

v4 design (vs the 332us f32r baseline):
- ALL matmuls run fp16 operands (fp32 PSUM accumulate); end-to-end rel
  err ~1.7e-3 vs the 2e-2 gate (fp8 simulated at 1.9-2.9% - rejected).
- The attention phase is ACT-bound (128 exps x ~1.14us); everything
  else is kept OFF the ACT engine during attention, and the PE's slack
  is filled by interleaving batch 1's QKV pair-groups plus the
  previous tqb's output projection into the attention emission stream
  (in-order engine queues = program order is the schedule). PE stalls
  also drop the HAM clock to 1.2GHz for whole 3.4us epochs, so a dense
  PE stream is worth double.
- PSUM budget (8 banks): scores 2x[128,1024] (4) + ctx accumulators
  2x[128,512] (2) + transient ring 2x[128,512] (2) shared by QKV
  projection pairs, v-transposes and outproj halves. The ctx
  accumulators are freed immediately after a craw/zrow SBUF copy; the
  1/Z chain (reciprocal_approx_fast + DRAM broadcast bounce) and the
  normalize-multiply are emitted a tqb later so their DMA latency
  hides under the exp stream.
- reciprocal_approx_fast must read its input from an SBUF tile at
  partition base 0 (PSUM or partition-base-64 input returns garbage -
  measured on hw).
- GpSimd runs the rope sin-mul/add, kZ copies, normalize-muls and half
  the outproj PSUM->SBUF copies; DVE takes the rest.
- Weights land pre-arranged from the host ([128, 8*128] fp16) so every
  weight DMA is contiguous; qT lives in 4 per-chunk tiles so tqb-0
  scores only depend on the first q chunk's RoPE.
- Attention matmuls stay zero-padded to full 128x128 array shapes
  (K=128 scores via zero-padded per-head K, M=128 ctx via padded
  augmented-V) - half-array matmuls don't register as activity for the
  PE's HAM clock gate and the whole phase runs at 1.2GHz otherwise.
"""

import numpy as np

B, S, H, NH, HD = 2, 2048, 1024, 16, 64
NCORES = 8
JC = 128  # head-columns per core (2 heads x 64)
T = B * S  # 4096 tokens
TQB = 512  # tq block
NKT = S // 128  # 16 tk blocks per batch
WAVE = 1024
BASE = 10000.0

_nc_cache = [None]

_LDW_OPT = False  # --enable-ldw-opt=true fails walrus codegen on our ldweights forms
_POOL_PSUM = False  # GPSIMD cannot access PSUM (BIR verifier)
_POOL_ROPE = True  # gpsimd does the rope sin-mul + add


def _patch_ldw_opt():
    from concourse import bass_utils as _bu

    if getattr(_bu, "_ldw_patched", False):
        return
    _orig = _bu.run_command

    def _patched(argv, **kw):
        argv = [
            a.replace("--enable-ldw-opt=false", "--enable-ldw-opt=true")
            if _LDW_OPT and isinstance(a, str)
            else a
            for a in argv
        ]
        return _orig(argv, **kw)

    _bu.run_command = _patched
    _bu._ldw_patched = True


def _build():
    _patch_ldw_opt()
    import concourse.tile as tile
    from concourse import bacc, mybir
    from concourse.masks import make_identity

    F32 = mybir.dt.float32
    F16 = mybir.dt.float16
    EXP = mybir.ActivationFunctionType.Exp

    nc = bacc.Bacc("TRN2", target_bir_lowering=False, debug=False)

    hsT = nc.dram_tensor("hsT", [H, T], F16, kind="ExternalInput").ap()
    wqP = nc.dram_tensor("wqP", [128, 8 * JC], F16, kind="ExternalInput").ap()
    wkP = nc.dram_tensor("wkP", [128, 8 * JC], F16, kind="ExternalInput").ap()
    wvP = nc.dram_tensor("wvP", [128, 8 * JC], F16, kind="ExternalInput").ap()
    woJI = nc.dram_tensor("woJI", [JC, H], F16, kind="ExternalInput").ap()
    cosT = nc.dram_tensor("cosT", [128, S], F16, kind="ExternalInput").ap()
    sinTs = nc.dram_tensor("sinTs", [128, S], F16, kind="ExternalInput").ap()
    out = nc.dram_tensor("out", [T, H], F16, kind="ExternalOutput").ap()

    with tile.TileContext(nc) as tc:
        with (
            tc.tile_pool(name="wts", bufs=1) as wts,
            tc.tile_pool(name="tabs", bufs=1) as tabs,
            tc.tile_pool(name="hst", bufs=36) as hst,
            tc.tile_pool(name="qkv", bufs=2) as qkvp,
            tc.tile_pool(name="ps", bufs=2, space="PSUM") as ps,
            tc.tile_pool(name="cxp", bufs=2, space="PSUM") as cxp,
            tc.tile_pool(name="rope", bufs=3) as ropep,
            tc.tile_pool(name="vaug", bufs=1) as vaugp,
            tc.tile_pool(name="expt", bufs=4) as exptp,
            tc.tile_pool(name="ctx", bufs=2) as ctxp,
            tc.tile_pool(name="nrm", bufs=4) as nrmp,
            tc.tile_pool(name="outs", bufs=4) as outsp,
            tc.tile_pool(name="zdr", bufs=4, space="DRAM") as zdrp,
        ):
            # ---- persistent weights / tables (all contiguous fp16 DMA) ----
            # wv first: the v chains run first and gate everything.
            wv_sb = wts.tile([128, 8, JC], F16, tag="wv")
            nc.sync.dma_start(
                out=wv_sb[:], in_=wvP[:, :].rearrange("p (k j) -> p k j", k=8)
            )
            wk_sb = wts.tile([128, 8, JC], F16, tag="wk")
            wq_sb = wts.tile([128, 8, JC], F16, tag="wq")
            cos_sb = tabs.tile([128, S], F16, tag="cos")
            sin_sb = tabs.tile([128, S], F16, tag="sin")
            wJ = wts.tile([128, H], F16, tag="wj")

            # preamble init runs on GpSimd so the DVE queue is free for the
            # first v-chain epilogues (DVE backlog here stalls the PE via the
            # transient-PSUM ring).
            onesc = tabs.tile([128, NKT], F16, tag="ones")
            nc.gpsimd.memset(onesc[:], 1.0)
            ident = tabs.tile([128, 128], F32, tag="ident")
            make_identity(nc, ident[:])

            # augmented-V and zero-padded-K buffers: one physical buffer
            # per batch parity, static zero/ones regions set once here.
            vAb, vBb, kZAb, kZBb = [], [], [], []
            for i in range(B):
                vA = vaugp.tile([128, NKT, 128], F16, tag=f"vA{i}")
                nc.gpsimd.memset(vA[:, :, 65:128], 0.0)
                nc.gpsimd.tensor_copy(vA[:, :, 64], onesc[:])
                vAb.append(vA)
                vB = vaugp.tile([128, NKT, 128], F16, tag=f"vB{i}")
                nc.gpsimd.memset(vB[:, :, 65:128], 0.0)
                nc.gpsimd.tensor_copy(vB[:, :, 64], onesc[:])
                vBb.append(vB)
                kZA = vaugp.tile([128, S], F16, tag=f"kZA{i}")
                nc.gpsimd.memset(kZA[64:128, :], 0.0)
                kZAb.append(kZA)
                kZB = vaugp.tile([128, S], F16, tag=f"kZB{i}")
                nc.gpsimd.memset(kZB[0:64, :], 0.0)
                kZBb.append(kZB)

            def make_qkv(b):
                """QKV projections (+RoPE) for batch b, emitted in pair-group
                steps via a generator so batch 1's groups can be interleaved
                into batch 0's (ACT-bound) attention emission. Keeps the ACT
                engine untouched: PSUM->SBUF copies go to DVE, the rope
                sin-mul/add to GpSimd."""
                vA, vB, kZA, kZB = vAb[b], vBb[b], kZAb[b], kZBb[b]
                qTc = [
                    qkvp.tile([128, TQB], F16, tag="qTc", name=f"qTc{b}_{i}", bufs=8)
                    for i in range(S // TQB)
                ]
                kT = qkvp.tile([128, S], F16, tag="kT", name=f"kT{b}")
                vT = qkvp.tile([128, S], F32, tag="vT", name=f"vT{b}")
                chunk_cache = {}

                def get_chunk(k, nchi):
                    if (k, nchi) not in chunk_cache:
                        t0 = b * S + nchi * TQB
                        c = hst.tile([128, TQB], F16, tag="hst", name="hst")
                        nc.sync.dma_start(
                            out=c[:], in_=hsT[128 * k : 128 * (k + 1), t0 : t0 + TQB]
                        )
                        chunk_cache[(k, nchi)] = c
                    return chunk_cache[(k, nchi)]

                def gen():
                    # v first (enables the v transposes early), then k, then q.
                    for kind, w_sb in (("v", wv_sb), ("k", wk_sb), ("q", wq_sb)):
                        for nch0 in range(0, S // TQB, 2):
                            pts = [
                                cxp.tile([128, TQB], F32, tag="tr", name="pt_a"),
                                cxp.tile([128, TQB], F32, tag="tr", name="pt_b"),
                            ]
                            for k in range(8):
                                for i in range(2):
                                    nc.tensor.matmul(
                                        pts[i][:],
                                        w_sb[:, k, :],
                                        get_chunk(k, nch0 + i)[:],
                                        start=(k == 0),
                                        stop=(k == 7),
                                    )
                            for i in range(2):
                                nchi = nch0 + i
                                sl = slice(nchi * TQB, (nchi + 1) * TQB)
                                p = pts[i]
                                if kind == "v":
                                    nc.vector.tensor_copy(vT[:, sl], p[:])
                                    # PE-transpose this chunk's 4 tk blocks
                                    # into one PSUM tile, then 2 batched
                                    # copies into the fp16 augmented-V layout.
                                    tp = cxp.tile(
                                        [128, TQB], F32, tag="tr", name="tp"
                                    )
                                    for j in range(TQB // 128):
                                        nc.tensor.transpose(
                                            tp[:, 128 * j : 128 * (j + 1)],
                                            vT[:, sl][:, 128 * j : 128 * (j + 1)],
                                            ident[:],
                                        )
                                    tpv = tp[:].rearrange("p (j c) -> p j c", j=4)
                                    t0 = nchi * (TQB // 128)
                                    nc.vector.tensor_copy(
                                        vA[:, t0 : t0 + 4, 0:64], tpv[:, :, 0:64]
                                    )
                                    nc.vector.tensor_copy(
                                        vB[:, t0 : t0 + 4, 0:64], tpv[:, :, 64:128]
                                    )
                                    continue
                                raw = ropep.tile([128, TQB], F16, tag="raw")
                                nc.vector.tensor_copy(raw[:], p[:])
                                rot = ropep.tile([128, TQB], F16, tag="rot")
                                for h0 in (0, 64):
                                    nc.sync.dma_start(
                                        out=rot[h0 : h0 + 32, :],
                                        in_=raw[h0 + 32 : h0 + 64, :],
                                    )
                                    nc.sync.dma_start(
                                        out=rot[h0 + 32 : h0 + 64, :],
                                        in_=raw[h0 : h0 + 32, :],
                                    )
                                t1 = ropep.tile([128, TQB], F16, tag="t1")
                                nc.vector.tensor_mul(t1[:], raw[:], cos_sb[:, sl])
                                t2 = ropep.tile([128, TQB], F16, tag="t2")
                                nc.gpsimd.tensor_mul(t2[:], rot[:], sin_sb[:, sl])
                                dst = qTc[nchi][:] if kind == "q" else kT[:, sl]
                                nc.vector.tensor_add(dst, t1[:], t2[:])
                            yield
                        if kind == "k":
                            nc.vector.tensor_copy(kZA[0:64, :], kT[0:64, :])
                            nc.vector.tensor_copy(kZB[64:128, :], kT[64:128, :])

                return gen(), qTc

            from collections import deque

            defer = deque()  # closures drained in later attention windows

            def attention(b, qTc, ileave, drain):
                """scoresT -> exp -> ctxT -> normalize. The p-loop emits, at
                fixed p slots: next batch's QKV pair-groups (p 2/5), the 1/Z
                finish + normalize-mul of the previous tqb (p 1), and deferred
                work (outproj units) at p 4/6. Batch 0 defers ALL its outproj
                into batch 1's (PE-slack) window; batch 1 defers only its last
                tqb's into the tail. The ACT engine sees nothing but the 16
                exps per tqb."""
                vA, vB, kZA, kZB = vAb[b], vBb[b], kZAb[b], kZBb[b]
                ctxS = ctxp.tile([128, S], F16, tag="cts", name=f"ctxS{b}")
                ctxB = ctxp.tile([64, S], F16, tag="ctb", name=f"ctxB{b}")
                pend = {}  # tqb -> (crawA, zrepA, crawB, zrepB)

                def norm_phase1(tqb, cxA, cxB):
                    # free the ctx PSUM accumulators ASAP; 1/Z + DRAM
                    # broadcast bounce starts before the craw copies so the
                    # DMA latency overlaps them.
                    res = []
                    for cx in (cxA, cxB):
                        zrow = nrmp.tile([1, TQB], F32, tag="zrow")
                        nc.vector.tensor_copy(zrow[:], cx[64:65, :])
                        rzf = nrmp.tile([1, TQB], F32, tag="rzf")
                        nc.vector.reciprocal_approx_fast(out=rzf[:], in_=zrow[:])
                        zd = zdrp.tile([1, TQB], F32, tag="zd")
                        nc.sync.dma_start(out=zd[:], in_=rzf[:])
                        zrep = nrmp.tile([64, TQB], F32, tag="zrep")
                        nc.sync.dma_start(
                            out=zrep[:], in_=zd[0:1, :].to_broadcast([64, TQB])
                        )
                        craw = nrmp.tile([64, TQB], F32, tag="craw")
                        nc.vector.tensor_copy(craw[:], cx[0:64, :])
                        res += [craw, zrep]
                    pend[tqb] = res

                def norm_phase2(tqb):
                    crawA, zrepA, crawB, zrepB = pend.pop(tqb)
                    qsl = slice(tqb * TQB, (tqb + 1) * TQB)
                    nc.gpsimd.tensor_mul(ctxS[0:64, qsl], crawA[:], zrepA[:])
                    nc.gpsimd.tensor_mul(ctxB[:, qsl], crawB[:], zrepB[:])
                    nc.scalar.dma_start(out=ctxS[64:128, qsl], in_=ctxB[:, qsl])

                def emit_outproj(tqb):
                    for j8 in range(TQB // 128):
                        tq8 = tqb * (TQB // 128) + j8
                        csl = slice(128 * tq8, 128 * (tq8 + 1))
                        poa = cxp.tile([128, TQB], F32, tag="tr", name="poa")
                        pob = cxp.tile([128, TQB], F32, tag="tr", name="pob")
                        nc.tensor.matmul(
                            poa[:], ctxS[:, csl], wJ[:, 0:512],
                            start=True, stop=True,
                        )
                        nc.tensor.matmul(
                            pob[:], ctxS[:, csl], wJ[:, 512:1024],
                            start=True, stop=True,
                        )
                        ot = outsp.tile([128, H], F16, tag="ot")
                        nc.vector.tensor_copy(ot[:, 0:512], poa[:])
                        nc.vector.tensor_copy(ot[:, 512:1024], pob[:])
                        nc.scalar.dma_start(
                            out=out[b * S + 128 * tq8 : b * S + 128 * (tq8 + 1), :],
                            in_=ot[:],
                        )

                prev_tqb = None
                for tqb in range(S // TQB):
                    cxA = cxp.tile([128, TQB], F32, tag="cx")
                    cxB = cxp.tile([128, TQB], F32, tag="cx")
                    for p in range(NKT // 2):
                        if p == 1 and prev_tqb is not None:
                            norm_phase2(prev_tqb)
                            defer.append(
                                lambda t=prev_tqb: emit_outproj(t)
                            )
                            prev_tqb = None
                        if (p == 2 or (p == 5 and tqb in (1, 2))) and (
                            ileave is not None
                        ):
                            next(ileave, None)
                        if p in (4, 6) and drain and defer:
                            defer.popleft()()
                        scA = ps.tile([128, 2 * TQB], F32, tag="ps")
                        scB = ps.tile([128, 2 * TQB], F32, tag="ps")
                        for t in range(2):
                            tkb = 2 * p + t
                            ksl = slice(128 * tkb, 128 * (tkb + 1))
                            nc.tensor.matmul(
                                scA[:, t * TQB : (t + 1) * TQB],
                                kZA[:, ksl], qTc[tqb][:],
                                start=True, stop=True,
                            )
                            nc.tensor.matmul(
                                scB[:, t * TQB : (t + 1) * TQB],
                                kZB[:, ksl], qTc[tqb][:],
                                start=True, stop=True,
                            )
                        etA = exptp.tile([128, 2 * TQB], F16, tag="et")
                        nc.scalar.activation(etA[:], scA[:], EXP, scale=0.125)
                        etB = exptp.tile([128, 2 * TQB], F16, tag="et")
                        nc.scalar.activation(etB[:], scB[:], EXP, scale=0.125)
                        for t in range(2):
                            tkb = 2 * p + t
                            st, sp = tkb == 0, tkb == NKT - 1
                            tsl = slice(t * TQB, (t + 1) * TQB)
                            nc.tensor.matmul(
                                cxA[:, :], vA[:, tkb, :], etA[:, tsl],
                                start=st, stop=sp,
                            )
                            nc.tensor.matmul(
                                cxB[:, :], vB[:, tkb, :], etB[:, tsl],
                                start=st, stop=sp,
                            )
                    norm_phase1(tqb, cxA, cxB)
                    prev_tqb = tqb
                # last tqb's normalize-finish + outproj go to the next window
                # (or the tail for the final batch).
                fin_tqb = prev_tqb

                def fin(t=fin_tqb):
                    norm_phase2(t)
                    emit_outproj(t)

                defer.append(fin)
                if ileave is not None:
                    for _ in ileave:
                        pass

            gen0, qTc0 = make_qkv(0)
            # queue the first v pair-group (and its chunk DMAs) before the
            # remaining weight/table loads so the PE starts ASAP.
            next(gen0, None)
            nc.sync.dma_start(
                out=wk_sb[:], in_=wkP[:, :].rearrange("p (k j) -> p k j", k=8)
            )
            nc.sync.dma_start(
                out=wq_sb[:], in_=wqP[:, :].rearrange("p (k j) -> p k j", k=8)
            )
            nc.scalar.dma_start(out=cos_sb[:], in_=cosT[:, :])
            nc.scalar.dma_start(out=sin_sb[:], in_=sinTs[:, :])
            nc.scalar.dma_start(out=wJ[:], in_=woJI[:, :])
            for _ in gen0:
                pass
            gen1, qTc1 = make_qkv(1)
            attention(0, qTc0, gen1, drain=False)
            attention(1, qTc1, None, drain=True)
            while defer:
                defer.popleft()()

    nc.compile()
    return nc


def _rope_tables():
    inv_freq = 1.0 / (BASE ** (np.arange(0, HD, 2, dtype=np.float64) / HD))
    t = np.arange(S, dtype=np.float64)
    freqs = np.outer(t, inv_freq)  # [S, 32]
    emb = np.concatenate([freqs, freqs], -1)  # [S, 64]
    cos = np.cos(emb).T.astype(np.float32)  # [64, S]
    sin = np.sin(emb).T.astype(np.float32)
    sin_signed = sin.copy()
    sin_signed[0:32] = -sin_signed[0:32]
    cosT = np.ascontiguousarray(np.tile(cos, (2, 1))).astype(np.float16)  # [128, S]
    sinTs = np.ascontiguousarray(np.tile(sin_signed, (2, 1))).astype(np.float16)
    return cosT, sinTs


def _make_in_maps(hidden_states, Wq, Wk, Wv, Wo):
    hsT = np.ascontiguousarray(
        hidden_states.astype(np.float32).reshape(T, H).T.astype(np.float16)
    )  # [H, T]
    cosT, sinTs = _rope_tables()

    def pack_w(Wx, sl):
        # SBUF layout [p, k, j]: partition p holds rows {k*128+p} of Wx[sl,:].T
        wT = Wx[sl, :].astype(np.float16).T  # [H, JC]
        return np.ascontiguousarray(
            wT.reshape(8, 128, JC).transpose(1, 0, 2).reshape(128, 8 * JC)
        )

    in_maps = []
    for c in range(NCORES):
        sl = slice(JC * c, JC * (c + 1))
        in_maps.append(
            {
                "hsT": hsT,
                "wqP": pack_w(Wq, sl),
                "wkP": pack_w(Wk, sl),
                "wvP": pack_w(Wv, sl),
                "woJI": np.ascontiguousarray(Wo[:, sl].astype(np.float16).T),
                "cosT": cosT,
                "sinTs": sinTs,
            }
        )
    return in_maps


def kernel(hidden_states, Wq, Wk, Wv, Wo):
    hidden_states = np.asarray(hidden_states, np.float32)
    Wq, Wk, Wv, Wo = (np.asarray(w, np.float32) for w in (Wq, Wk, Wv, Wo))

    if _nc_cache[0] is None:
        _nc_cache[0] = _build()
    nc = _nc_cache[0]

    in_maps = _make_in_maps(hidden_states, Wq, Wk, Wv, Wo)

    from concourse.bass_utils import run_bass_kernel_spmd

    res = run_bass_kernel_spmd(nc, in_maps, core_ids=list(range(NCORES)))
    acc = np.zeros((T, H), np.float32)
    for c in range(NCORES):
        acc += res.results[c]["out"].astype(np.float32)
    return acc.reshape(B, S, H)


# revision 32
# speedup vs baseline: 1.2666x; 1.0196x over previous
"""Multi-head attention (B=2, S=2048, H=1024, NH=16, HD=64) on 8 trn2 cores.

Sharding: tensor-parallel over heads. Core c owns heads {2c, 2c+1}, i.e.
feature columns [128c, 128c+128) of q/k/v. Wq/Wk/Wv are column-sharded,
Wo row-sharded; each core computes a full-shape partial output and the
host sums the 8 partials (the row-parallel reduce) during unshard.

On-chip layout is feature-major ("transposed"): the host passes
hsT = hidden_states.T so both matmul operands of every projection have
the contraction dim on partitions and no on-chip transposes of big
tensors are needed. Attention works on scoresT[tk, tq]; softmax's
normalizer comes from a ones-column augmented V matmul (exp is safe
without max-subtraction because scores are O(9) here).

v4 design (vs the 332us f32r baseline):
- ALL matmuls run fp16 operands (fp32 PSUM accumulate); end-to-end rel
  err ~1.7e-3 vs the 2e-2 gate (fp8 simulated at 1.9-2.9% - rejected).
- The attention phase is ACT-bound (128 exps x ~1.14us); everything
  else is kept OFF the ACT engine during attention, and the PE's slack
  is filled by interleaving batch 1's QKV pair-groups plus the
  previous tqb's output projection into the attention emission stream
  (in-order engine queues = program order is the schedule). PE stalls
  also drop the HAM clock to 1.2GHz for whole 3.4us epochs, so a dense
  PE stream is worth double.
- PSUM budget (8 banks): scores 2x[128,1024] (4) + ctx accumulators
  2x[128,512] (2) + transient ring 2x[128,512] (2) shared by QKV
  projection pairs, v-transposes and outproj halves. The ctx
  accumulators are freed immediately after a craw/zrow SBUF copy; the
  1/Z chain (reciprocal_approx_fast + DRAM broadcast bounce) and the
  normalize-multiply are emitted a tqb later so their DMA latency
  hides under the exp stream.
- reciprocal_approx_fast must read its input from an SBUF tile at
  partition base 0 (PSUM or partition-base-64 input returns garbage -
  measured on hw).
- GpSimd runs the rope sin-mul/add, kZ copies, normalize-muls and half
  the outproj PSUM->SBUF copies; DVE takes the rest.
- Weights land pre-arranged from the host ([128, 8*128] fp16) so every
  weight DMA is contiguous; qT lives in 4 per-chunk tiles so tqb-0
  scores only depend on the first q chunk's RoPE.
- Attention matmuls stay zero-padded to full 128x128 array shapes
  (K=128 scores via zero-padded per-head K, M=128 ctx via padded
  augmented-V) - half-array matmuls don't register as activity for the
  PE's HAM clock gate and the whole phase runs at 1.2GHz otherwise.
"""

import numpy as np

B, S, H, NH, HD = 2, 2048, 1024, 16, 64
NCORES = 8
JC = 128  # head-columns per core (2 heads x 64)
T = B * S  # 4096 tokens
TQB = 512  # tq block
NKT = S // 128  # 16 tk blocks per batch
WAVE = 1024
BASE = 10000.0

_nc_cache = [None]

_LDW_OPT = False  # --enable-ldw-opt=true fails walrus codegen on our ldweights forms
_POOL_PSUM = False  # GPSIMD cannot access PSUM (BIR verifier)
_POOL_ROPE = True  # gpsimd does the rope sin-mul + add


def _patch_ldw_opt():
    from concourse import bass_utils as _bu

    if getattr(_bu, "_ldw_patched", False):
        return
    _orig = _bu.run_command

    def _patched(argv, **kw):
        argv = [
            a.replace("--enable-ldw-opt=false", "--enable-ldw-opt=true")
            if _LDW_OPT and isinstance(a, str)
            else a
            for a in argv
        ]
        return _orig(argv, **kw)

    _bu.run_command = _patched
    _bu._ldw_patched = True


def _build():
    _patch_ldw_opt()
    import concourse.tile as tile
    from concourse import bacc, mybir
    from concourse.masks import make_identity

    F32 = mybir.dt.float32
    F16 = mybir.dt.float16
    EXP = mybir.ActivationFunctionType.Exp

    nc = bacc.Bacc("TRN2", target_bir_lowering=False, debug=False)

    hsT = nc.dram_tensor("hsT", [H, T], F16, kind="ExternalInput").ap()
    wqP = nc.dram_tensor("wqP", [128, 8 * JC], F16, kind="ExternalInput").ap()
    wkP = nc.dram_tensor("wkP", [128, 8 * JC], F16, kind="ExternalInput").ap()
    wvP = nc.dram_tensor("wvP", [128, 8 * JC], F16, kind="ExternalInput").ap()
    woJI = nc.dram_tensor("woJI", [JC, H], F16, kind="ExternalInput").ap()
    cosT = nc.dram_tensor("cosT", [128, S], F16, kind="ExternalInput").ap()
    sinTs = nc.dram_tensor("sinTs", [128, S], F16, kind="ExternalInput").ap()
    out = nc.dram_tensor("out", [T, H], F16, kind="ExternalOutput").ap()

    with tile.TileContext(nc) as tc:
        with (
            tc.tile_pool(name="wts", bufs=1) as wts,
            tc.tile_pool(name="tabs", bufs=1) as tabs,
            tc.tile_pool(name="hst", bufs=36) as hst,
            tc.tile_pool(name="qkv", bufs=2) as qkvp,
            tc.tile_pool(name="ps", bufs=2, space="PSUM") as ps,
            tc.tile_pool(name="cxp", bufs=2, space="PSUM") as cxp,
            tc.tile_pool(name="rope", bufs=3) as ropep,
            tc.tile_pool(name="vaug", bufs=1) as vaugp,
            tc.tile_pool(name="expt", bufs=4) as exptp,
            tc.tile_pool(name="ctx", bufs=2) as ctxp,
            tc.tile_pool(name="nrm", bufs=4) as nrmp,
            tc.tile_pool(name="outs", bufs=4) as outsp,
            tc.tile_pool(name="zdr", bufs=4, space="DRAM") as zdrp,
        ):
            # ---- persistent weights / tables (all contiguous fp16 DMA) ----
            # wv first: the v chains run first and gate everything.
            wv_sb = wts.tile([128, 8, JC], F16, tag="wv")
            nc.sync.dma_start(
                out=wv_sb[:], in_=wvP[:, :].rearrange("p (k j) -> p k j", k=8)
            )
            wk_sb = wts.tile([128, 8, JC], F16, tag="wk")
            wq_sb = wts.tile([128, 8, JC], F16, tag="wq")
            cos_sb = tabs.tile([128, S], F16, tag="cos")
            sin_sb = tabs.tile([128, S], F16, tag="sin")
            wJ = wts.tile([128, H], F16, tag="wj")

            # preamble init runs on GpSimd so the DVE queue is free for the
            # first v-chain epilogues (DVE backlog here stalls the PE via the
            # transient-PSUM ring).
            onesc = tabs.tile([128, NKT], F16, tag="ones")
            nc.gpsimd.memset(onesc[:], 1.0)
            onecol = tabs.tile([1, 64], F32, tag="onecol")
            nc.gpsimd.memset(onecol[:], 1.0)
            ident = tabs.tile([128, 128], F32, tag="ident")
            make_identity(nc, ident[:])

            # augmented-V and zero-padded-K buffers: one physical buffer
            # per batch parity, static zero/ones regions set once here.
            vAb, vBb, kZAb, kZBb = [], [], [], []
            for i in range(B):
                vA = vaugp.tile([128, NKT, 128], F16, tag=f"vA{i}")
                nc.gpsimd.memset(vA[:, :, 65:128], 0.0)
                nc.gpsimd.tensor_copy(vA[:, :, 64], onesc[:])
                vAb.append(vA)
                vB = vaugp.tile([128, NKT, 128], F16, tag=f"vB{i}")
                nc.gpsimd.memset(vB[:, :, 65:128], 0.0)
                nc.gpsimd.tensor_copy(vB[:, :, 64], onesc[:])
                vBb.append(vB)
                kZA = vaugp.tile([128, S], F16, tag=f"kZA{i}")
                nc.gpsimd.memset(kZA[64:128, :], 0.0)
                kZAb.append(kZA)
                kZB = vaugp.tile([128, S], F16, tag=f"kZB{i}")
                nc.gpsimd.memset(kZB[0:64, :], 0.0)
                kZBb.append(kZB)

            def make_qkv(b, tp_tag):
                """QKV projections (+RoPE) for batch b, emitted in pair-group
                steps via a generator so batch 1's groups can be interleaved
                into batch 0's (ACT-bound) attention emission. Keeps the ACT
                engine untouched: PSUM->SBUF copies go to DVE, the rope
                sin-mul/add to GpSimd."""
                vA, vB, kZA, kZB = vAb[b], vBb[b], kZAb[b], kZBb[b]
                qTc = [
                    qkvp.tile([128, TQB], F16, tag="qTc", name=f"qTc{b}_{i}", bufs=8)
                    for i in range(S // TQB)
                ]
                kT = qkvp.tile([128, S], F16, tag="kT", name=f"kT{b}")
                vT = qkvp.tile([128, S], F32, tag="vT", name=f"vT{b}")
                chunk_cache = {}

                def get_chunk(k, nchi):
                    if (k, nchi) not in chunk_cache:
                        t0 = b * S + nchi * TQB
                        c = hst.tile([128, TQB], F16, tag="hst", name="hst")
                        nc.sync.dma_start(
                            out=c[:], in_=hsT[128 * k : 128 * (k + 1), t0 : t0 + TQB]
                        )
                        chunk_cache[(k, nchi)] = c
                    return chunk_cache[(k, nchi)]

                def gen():
                    # v first (enables the v transposes early), then k, then q.
                    for kind, w_sb in (("v", wv_sb), ("k", wk_sb), ("q", wq_sb)):
                        for nch0 in range(0, S // TQB, 2):
                            pts = [
                                cxp.tile([128, TQB], F32, tag="tr", name="pt_a"),
                                cxp.tile([128, TQB], F32, tag="tr", name="pt_b"),
                            ]
                            for k in range(8):
                                for i in range(2):
                                    nc.tensor.matmul(
                                        pts[i][:],
                                        w_sb[:, k, :],
                                        get_chunk(k, nch0 + i)[:],
                                        start=(k == 0),
                                        stop=(k == 7),
                                    )
                            for i in range(2):
                                nchi = nch0 + i
                                sl = slice(nchi * TQB, (nchi + 1) * TQB)
                                p = pts[i]
                                if kind == "v":
                                    nc.vector.tensor_copy(vT[:, sl], p[:])
                                    # PE-transpose this chunk's 4 tk blocks
                                    # into one PSUM tile, then 2 batched
                                    # copies into the fp16 augmented-V layout.
                                    tp = cxp.tile(
                                        [128, TQB], F32, tag=tp_tag, name="tp"
                                    )
                                    for j in range(TQB // 128):
                                        nc.tensor.transpose(
                                            tp[:, 128 * j : 128 * (j + 1)],
                                            vT[:, sl][:, 128 * j : 128 * (j + 1)],
                                            ident[:],
                                        )
                                    tpv = tp[:].rearrange("p (j c) -> p j c", j=4)
                                    t0 = nchi * (TQB // 128)
                                    nc.vector.tensor_copy(
                                        vA[:, t0 : t0 + 4, 0:64], tpv[:, :, 0:64]
                                    )
                                    nc.vector.tensor_copy(
                                        vB[:, t0 : t0 + 4, 0:64], tpv[:, :, 64:128]
                                    )
                                    continue
                                raw = ropep.tile([128, TQB], F16, tag="raw")
                                nc.vector.tensor_copy(raw[:], p[:])
                                rot = ropep.tile([128, TQB], F16, tag="rot")
                                for h0 in (0, 64):
                                    nc.sync.dma_start(
                                        out=rot[h0 : h0 + 32, :],
                                        in_=raw[h0 + 32 : h0 + 64, :],
                                    )
                                    nc.sync.dma_start(
                                        out=rot[h0 + 32 : h0 + 64, :],
                                        in_=raw[h0 : h0 + 32, :],
                                    )
                                t1 = ropep.tile([128, TQB], F16, tag="t1")
                                nc.vector.tensor_mul(t1[:], raw[:], cos_sb[:, sl])
                                t2 = ropep.tile([128, TQB], F16, tag="t2")
                                nc.gpsimd.tensor_mul(t2[:], rot[:], sin_sb[:, sl])
                                dst = qTc[nchi][:] if kind == "q" else kT[:, sl]
                                nc.vector.tensor_add(dst, t1[:], t2[:])
                            yield
                        if kind == "k":
                            nc.vector.tensor_copy(kZA[0:64, :], kT[0:64, :])
                            nc.vector.tensor_copy(kZB[64:128, :], kT[64:128, :])

                return gen(), qTc

            from collections import deque

            defer = deque()  # closures drained in later attention windows

            def attention(b, qTc, ileave, drain, tail=False):
                """scoresT -> exp -> ctxT -> normalize. The p-loop emits, at
                fixed p slots: next batch's QKV pair-groups (p 2/5), the 1/Z
                finish + normalize-mul of the previous tqb (p 1), and deferred
                work (outproj units) at p 4/6. Batch 0 defers ALL its outproj
                into batch 1's (PE-slack) window; batch 1 defers only its last
                tqb's into the tail. The ACT engine sees nothing but the 16
                exps per tqb."""
                vA, vB, kZA, kZB = vAb[b], vBb[b], kZAb[b], kZBb[b]
                ctxS = ctxp.tile([128, S], F16, tag="cts", name=f"ctxS{b}")
                ctxB = ctxp.tile([64, S], F16, tag="ctb", name=f"ctxB{b}")
                pend = {}  # tqb -> (crawA, zrepA, crawB, zrepB)

                def norm_phase1(tqb, cxA, cxB, last=False):
                    # free the ctx PSUM accumulators ASAP; 1/Z + DRAM
                    # broadcast bounce starts before the craw copies so the
                    # DMA latency overlaps them. For the tail (last=True) the
                    # bounce is skipped - fin_tail broadcasts 1/Z via the PE.
                    res = []
                    for cx in (cxA, cxB):
                        zrow = nrmp.tile([1, TQB], F32, tag="zrow")
                        nc.vector.tensor_copy(zrow[:], cx[64:65, :])
                        rzf = nrmp.tile([1, TQB], F32, tag="rzf")
                        nc.vector.reciprocal_approx_fast(out=rzf[:], in_=zrow[:])
                        if last:
                            rep = rzf
                        else:
                            zd = zdrp.tile([1, TQB], F32, tag="zd")
                            nc.sync.dma_start(out=zd[:], in_=rzf[:])
                            rep = nrmp.tile([64, TQB], F32, tag="zrep")
                            nc.sync.dma_start(
                                out=rep[:], in_=zd[0:1, :].to_broadcast([64, TQB])
                            )
                        craw = nrmp.tile([64, TQB], F32, tag="craw")
                        nc.vector.tensor_copy(craw[:], cx[0:64, :])
                        res += [craw, rep]
                    pend[tqb] = res

                def norm_phase2(tqb):
                    crawA, zrepA, crawB, zrepB = pend.pop(tqb)
                    qsl = slice(tqb * TQB, (tqb + 1) * TQB)
                    nc.gpsimd.tensor_mul(ctxS[0:64, qsl], crawA[:], zrepA[:])
                    nc.gpsimd.tensor_mul(ctxB[:, qsl], crawB[:], zrepB[:])
                    nc.scalar.dma_start(out=ctxS[64:128, qsl], in_=ctxB[:, qsl])

                def emit_outproj(tqb):
                    for j8 in range(TQB // 128):
                        tq8 = tqb * (TQB // 128) + j8
                        csl = slice(128 * tq8, 128 * (tq8 + 1))
                        poa = cxp.tile([128, TQB], F32, tag="tr", name="poa")
                        pob = cxp.tile([128, TQB], F32, tag="tr", name="pob")
                        nc.tensor.matmul(
                            poa[:], ctxS[:, csl], wJ[:, 0:512],
                            start=True, stop=True,
                        )
                        nc.tensor.matmul(
                            pob[:], ctxS[:, csl], wJ[:, 512:1024],
                            start=True, stop=True,
                        )
                        ot = outsp.tile([128, H], F16, tag="ot")
                        nc.vector.tensor_copy(ot[:, 0:512], poa[:])
                        nc.vector.tensor_copy(ot[:, 512:1024], pob[:])
                        nc.scalar.dma_start(
                            out=out[b * S + 128 * tq8 : b * S + 128 * (tq8 + 1), :],
                            in_=ot[:],
                        )

                prev_tqb = None
                for tqb in range(S // TQB):
                    cxA = cxp.tile([128, TQB], F32, tag="cx")
                    cxB = cxp.tile([128, TQB], F32, tag="cx")
                    for p in range(NKT // 2):
                        if p == 1 and prev_tqb is not None:
                            norm_phase2(prev_tqb)
                            defer.append(
                                lambda t=prev_tqb: emit_outproj(t)
                            )
                            prev_tqb = None
                        if (p == 2 or (p == 5 and tqb in (1, 2))) and (
                            ileave is not None
                        ):
                            next(ileave, None)
                        if p in (4, 6) and drain and defer:
                            defer.popleft()()
                        scA = ps.tile([128, 2 * TQB], F32, tag="ps")
                        scB = ps.tile([128, 2 * TQB], F32, tag="ps")
                        for t in range(2):
                            tkb = 2 * p + t
                            ksl = slice(128 * tkb, 128 * (tkb + 1))
                            nc.tensor.matmul(
                                scA[:, t * TQB : (t + 1) * TQB],
                                kZA[:, ksl], qTc[tqb][:],
                                start=True, stop=True,
                            )
                            nc.tensor.matmul(
                                scB[:, t * TQB : (t + 1) * TQB],
                                kZB[:, ksl], qTc[tqb][:],
                                start=True, stop=True,
                            )
                        etA = exptp.tile([128, 2 * TQB], F16, tag="et")
                        nc.scalar.activation(etA[:], scA[:], EXP, scale=0.125)
                        etB = exptp.tile([128, 2 * TQB], F16, tag="et")
                        nc.scalar.activation(etB[:], scB[:], EXP, scale=0.125)
                        for t in range(2):
                            tkb = 2 * p + t
                            st, sp = tkb == 0, tkb == NKT - 1
                            tsl = slice(t * TQB, (t + 1) * TQB)
                            nc.tensor.matmul(
                                cxA[:, :], vA[:, tkb, :], etA[:, tsl],
                                start=st, stop=sp,
                            )
                            nc.tensor.matmul(
                                cxB[:, :], vB[:, tkb, :], etB[:, tsl],
                                start=st, stop=sp,
                            )
                    norm_phase1(
                        tqb, cxA, cxB, last=tail and tqb == S // TQB - 1
                    )
                    prev_tqb = tqb
                # last tqb's normalize-finish + outproj go to the next window
                # (or the tail for the final batch).
                fin_tqb = prev_tqb

                def fin(t=fin_tqb):
                    norm_phase2(t)
                    emit_outproj(t)

                def fin_tail(t=fin_tqb):
                    # latency-optimized tail: PE ones-matmul broadcast of 1/Z
                    # (no DRAM bounce), norm-muls on DVE, outproj copies split
                    # DVE/ACT (both idle by now).
                    crawA, rzfA, crawB, rzfB = pend.pop(t)
                    qsl = slice(t * TQB, (t + 1) * TQB)
                    zpsA = cxp.tile([128, TQB], F32, tag="tr", name="zpsA")
                    nc.tensor.matmul(
                        zpsA[0:64, :], onecol[:], rzfA[:], start=True, stop=True
                    )
                    zpsB = cxp.tile([128, TQB], F32, tag="tr", name="zpsB")
                    nc.tensor.matmul(
                        zpsB[0:64, :], onecol[:], rzfB[:], start=True, stop=True
                    )
                    nc.vector.tensor_mul(ctxS[0:64, qsl], crawA[:], zpsA[0:64, :])
                    nc.vector.tensor_mul(ctxB[:, qsl], crawB[:], zpsB[0:64, :])
                    nc.scalar.dma_start(out=ctxS[64:128, qsl], in_=ctxB[:, qsl])
                    for j8 in range(TQB // 128):
                        tq8 = t * (TQB // 128) + j8
                        csl = slice(128 * tq8, 128 * (tq8 + 1))
                        poa = cxp.tile([128, TQB], F32, tag="tr", name="poa")
                        pob = cxp.tile([128, TQB], F32, tag="tr", name="pob")
                        nc.tensor.matmul(
                            poa[:], ctxS[:, csl], wJ[:, 0:512],
                            start=True, stop=True,
                        )
                        nc.tensor.matmul(
                            pob[:], ctxS[:, csl], wJ[:, 512:1024],
                            start=True, stop=True,
                        )
                        ot = outsp.tile([128, H], F16, tag="ot")
                        nc.vector.tensor_copy(ot[:, 0:512], poa[:])
                        nc.scalar.copy(ot[:, 512:1024], pob[:])
                        nc.scalar.dma_start(
                            out=out[b * S + 128 * tq8 : b * S + 128 * (tq8 + 1), :],
                            in_=ot[:],
                        )

                defer.append(fin_tail if tail else fin)
                if ileave is not None:
                    for _ in ileave:
                        pass

            gen0, qTc0 = make_qkv(0, tp_tag="cx")
            # queue the first v pair-group (and its chunk DMAs) before the
            # remaining weight/table loads so the PE starts ASAP.
            next(gen0, None)
            nc.sync.dma_start(
                out=wk_sb[:], in_=wkP[:, :].rearrange("p (k j) -> p k j", k=8)
            )
            nc.sync.dma_start(
                out=wq_sb[:], in_=wqP[:, :].rearrange("p (k j) -> p k j", k=8)
            )
            nc.scalar.dma_start(out=cos_sb[:], in_=cosT[:, :])
            nc.scalar.dma_start(out=sin_sb[:], in_=sinTs[:, :])
            nc.scalar.dma_start(out=wJ[:], in_=woJI[:, :])
            for _ in gen0:
                pass
            gen1, qTc1 = make_qkv(1, tp_tag="tr")
            attention(0, qTc0, gen1, drain=False)
            attention(1, qTc1, None, drain=True, tail=True)
            while defer:
                defer.popleft()()

    nc.compile()
    return nc


def _rope_tables():
    inv_freq = 1.0 / (BASE ** (np.arange(0, HD, 2, dtype=np.float64) / HD))
    t = np.arange(S, dtype=np.float64)
    freqs = np.outer(t, inv_freq)  # [S, 32]
    emb = np.concatenate([freqs, freqs], -1)  # [S, 64]
    cos = np.cos(emb).T.astype(np.float32)  # [64, S]
    sin = np.sin(emb).T.astype(np.float32)
    sin_signed = sin.copy()
    sin_signed[0:32] = -sin_signed[0:32]
    cosT = np.ascontiguousarray(np.tile(cos, (2, 1))).astype(np.float16)  # [128, S]
    sinTs = np.ascontiguousarray(np.tile(sin_signed, (2, 1))).astype(np.float16)
    return cosT, sinTs


def _make_in_maps(hidden_states, Wq, Wk, Wv, Wo):
    hsT = np.ascontiguousarray(
        hidden_states.astype(np.float32).reshape(T, H).T.astype(np.float16)
    )  # [H, T]
    cosT, sinTs = _rope_tables()

    def pack_w(Wx, sl):
        # SBUF layout [p, k, j]: partition p holds rows {k*128+p} of Wx[sl,:].T
        wT = Wx[sl, :].astype(np.float16).T  # [H, JC]
        return np.ascontiguousarray(
            wT.reshape(8, 128, JC).transpose(1, 0, 2).reshape(128, 8 * JC)
        )

    in_maps = []
    for c in range(NCORES):
        sl = slice(JC * c, JC * (c + 1))
        in_maps.append(
            {
                "hsT": hsT,
                "wqP": pack_w(Wq, sl),
                "wkP": pack_w(Wk, sl),
                "wvP": pack_w(Wv, sl),
                "woJI": np.ascontiguousarray(Wo[:, sl].astype(np.float16).T),
                "cosT": cosT,
                "sinTs": sinTs,
            }
        )
    return in_maps


def kernel(hidden_states, Wq, Wk, Wv, Wo):
    hidden_states = np.asarray(hidden_states, np.float32)
    Wq, Wk, Wv, Wo = (np.asarray(w, np.float32) for w in (Wq, Wk, Wv, Wo))

    if _nc_cache[0] is None:
        _nc_cache[0] = _build()
    nc = _nc_cache[0]

    in_maps = _make_in_maps(hidden_states, Wq, Wk, Wv, Wo)

    from concourse.bass_utils import run_bass_kernel_spmd

    res = run_bass_kernel_spmd(nc, in_maps, core_ids=list(range(NCORES)))
    acc = np.zeros((T, H), np.float32)
    for c in range(NCORES):
        acc += res.results[c]["out"].astype(np.float32)
    return acc.reshape(B, S, H)
